# revision 1
# baseline (speedup 1.0000x reference)
"""Trainium2 Bass kernel for nn_AttentionBlock (MLA-style attention + SwiGLU FFN).

Self-contained: takes FULL inputs, shards across 8 NeuronCores internally,
returns FULL output.

Sharding:
  Launch 1 (attention): tensor-parallel over heads (2 heads/core); each core
    computes its heads' partial attn_out @ W_O slice; host sums partials.
  Launch 2 (FFN): 2D sharding (4 token-quarters x 2 ffn-halves); host sums
    the two ffn-half partials per token quarter.
All matmuls run in bf16 with fp32 PSUM accumulation. Softmax/normalization
statistics are computed in fp32. Activations arrive pre-transposed (host does
the [m,D]->[D,m] transpose), so the device never transposes.
"""
import sys
sys.path.insert(0, '/opt/trn_rl_repo')

import math
import numpy as np
import ml_dtypes

from concourse import bass, bacc, mybir, tile
from concourse.bass_utils import run_bass_kernel_spmd

# ---- inlined wait pruner (kernel.py must be self-contained) ----
import bisect


def _is_dma(inst):
    return type(inst).__name__ in (
        "InstDMACopy", "InstDmaTranspose", "InstDmaTransposeAnt",
        "InstTensorCopyDma", "InstTensorReduceDma")


def prune_redundant_waits(nc, verbose=False):
    insts = []
    for f in nc.m.functions:
        for blk in f.blocks:
            insts.extend(blk.instructions)

    poisoned = set()
    running = {}
    producers = {}   # sem -> ([values], [idx])
    VC = [None] * len(insts)
    chain_vc = {}    # engine -> completion vc of last instruction
    chain_prev = [None] * len(insts)   # vc inherited from chain (pre-wait)

    def producer_at_least(sem, v):
        if sem in poisoned or sem not in producers:
            return None
        vals, idxs = producers[sem]
        i = bisect.bisect_left(vals, v)
        if i == len(vals):
            return None
        return vals[i], idxs[i]

    def merge(dst, src):
        for s, v in src.items():
            if dst.get(s, -1) < v:
                dst[s] = v

    for idx, inst in enumerate(insts):
        si = inst.sync_info
        is_dma = _is_dma(inst)
        ekey = getattr(inst, "engine", None)
        if is_dma:
            vc = {}
        else:
            vc = dict(chain_vc.get(ekey, {}))
        chain_prev[idx] = dict(vc)
        if si is not None:
            for w in si.on_wait:
                if w.wait_mode != "sem-ge-imm" or w.id in poisoned:
                    continue
                p = producer_at_least(w.id, w.wait_value)
                if p is not None:
                    merge(vc, VC[p[1]])
                    if vc.get(w.id, -1) < p[0]:
                        vc[w.id] = p[0]
                else:
                    if vc.get(w.id, -1) < w.wait_value:
                        vc[w.id] = w.wait_value
            for u in si.on_update:
                if u.update_mode in ("sem-inc", "sem-add-imm"):
                    nv = running.get(u.id, 0) + u.update_value
                    running[u.id] = nv
                    producers.setdefault(u.id, ([], []))
                    producers[u.id][0].append(nv)
                    producers[u.id][1].append(idx)
                    if vc.get(u.id, -1) < nv:
                        vc[u.id] = nv
                else:
                    poisoned.add(u.id)
        VC[idx] = vc
        if not is_dma:
            chain_vc[ekey] = vc

    # pass 2: prune
    n_pruned = 0
    for idx, inst in enumerate(insts):
        si = inst.sync_info
        if si is None or len(si.on_wait) < 2:
            continue
        waits = list(si.on_wait)
        kept = list(waits)
        changed = True
        while changed and len(kept) > 1:
            changed = False
            for w in kept:
                if w.wait_mode != "sem-ge-imm" or w.id in poisoned:
                    continue
                cover = dict(chain_prev[idx])
                ok_others = True
                for o in kept:
                    if o is w:
                        continue
                    if o.wait_mode != "sem-ge-imm" or o.id in poisoned:
                        continue
                    p = producer_at_least(o.id, o.wait_value)
                    if p is not None:
                        merge(cover, VC[p[1]])
                if cover.get(w.id, -1) >= w.wait_value:
                    kept.remove(w)
                    n_pruned += 1
                    changed = True
                    break
        if len(kept) != len(waits):
            import concourse.mybir as mybir
            inst.sync_info = mybir.SyncInfo(on_wait=kept, on_update=list(si.on_update))
    if verbose:
        hist = {}
        for inst in insts:
            si = inst.sync_info
            n = len(si.on_wait) if si else 0
            k = (type(inst).__name__, n)
            hist[k] = hist.get(k, 0) + 1
        print(f"wait_prune: removed {n_pruned} waits; post histogram:",
              dict(sorted(hist.items())))
    return n_pruned

# ---- end wait pruner ----


BF = mybir.dt.bfloat16
F16 = mybir.dt.float16
F32 = mybir.dt.float32
AF = mybir.ActivationFunctionType

D = 2048
N_H = 16
D_H = 128
D_R = 64
FFN = 8192
THETA = 1000000.0
EPS = 1e-6
SCALE = 1.0 / math.sqrt(D_H + D_R)
NCORES = 8
P = 128
MB = 512


# --------------------------------------------------------------------------
# Launch 1: attention block, tensor-parallel over heads
# --------------------------------------------------------------------------
def build_attn(B, M, N, Dm, HPC, DH=D_H, DR=D_R):
    DC = Dm // P
    NT = N // P
    MT = M // P
    NBN = N // MB
    NBM = M // MB
    RD = HPC * DR
    HD = HPC * DH
    ln_scale_bias = float(math.log(SCALE))

    nc = bacc.Bacc()
    qT = nc.dram_tensor("qT", [B, Dm, M], BF, kind="ExternalInput")
    kvT = nc.dram_tensor("kvT", [B, Dm, N], BF, kind="ExternalInput")
    wq = nc.dram_tensor("wq", [Dm, HD], BF, kind="ExternalInput")
    wqr = nc.dram_tensor("wqr", [Dm, RD], BF, kind="ExternalInput")
    wk = nc.dram_tensor("wk", [Dm, HD], BF, kind="ExternalInput")
    wkr = nc.dram_tensor("wkr", [Dm, RD], BF, kind="ExternalInput")
    wv = nc.dram_tensor("wv", [Dm, HD], BF, kind="ExternalInput")
    wo = nc.dram_tensor("wo", [HD, Dm], BF, kind="ExternalInput")
    cos2T = nc.dram_tensor("cos2T", [RD, M], F16, kind="ExternalInput")
    sin2T = nc.dram_tensor("sin2T", [RD, M], F16, kind="ExternalInput")
    rot2T = nc.dram_tensor("rot2T", [RD, RD], BF, kind="ExternalInput")
    po = nc.dram_tensor("po", [B, M, Dm], F32, kind="ExternalOutput")

    with tile.TileContext(nc) as tc:
      with tc.tile_pool(name="const", bufs=1) as cp, \
           tc.tile_pool(name="dram", bufs=1, space="DRAM") as dramp:
        ones_bf = cp.tile([P, 1], BF, tag="ones")
        nc.vector.memset(ones_bf[:], 1.0)
        cosT_sb = cp.tile([RD, M], F16, tag="cos")
        sinT_sb = cp.tile([RD, M], F16, tag="sin")
        rot_sb = cp.tile([RD, RD], BF, tag="rot")
        eps_t = cp.tile([P, 1], F32, tag="eps")
        nc.vector.memset(eps_t[:], EPS)
        lnsc_t = cp.tile([P, 1], F32, tag="lnsc")
        nc.vector.memset(lnsc_t[:], ln_scale_bias)
        nc.sync.dma_start(out=cosT_sb[:], in_=cos2T[:])
        nc.sync.dma_start(out=sinT_sb[:], in_=sin2T[:])
        nc.sync.dma_start(out=rot_sb[:], in_=rot2T[:])

        for b in range(B):
          with tc.tile_pool(name=f"kq{b}", bufs=1) as kq:
            kt = [kq.tile([P, N], BF, tag=f"kt{h}", name=f"kt{h}") for h in range(HPC)]
            krt = kq.tile([RD, N], BF, tag="krt")
            vt = [kq.tile([P, HD], BF, tag=f"vt{i}", name=f"vt{i}") for i in range(NT)]
            qt = [kq.tile([P, M], BF, tag=f"qt{h}", name=f"qt{h}") for h in range(HPC)]
            qrt = kq.tile([RD, M], BF, tag="qrt")
            nkv_col = kq.tile([P, NT], F32, tag="nkvc")
            nkvV_col = kq.tile([P, NT], F32, tag="nkvvc")
            nq_bc = kq.tile([P, M], F32, tag="nqbc")

            # ================= KV side =================
            with tc.tile_pool(name=f"kvw{b}", bufs=1) as wp, \
                 tc.tile_pool(name=f"kvs{b}", bufs=2) as sp:
                wkt = [wp.tile([P, HD], BF, tag=f"wk{i}", name=f"wk{i}") for i in range(DC)]
                wkrt = [wp.tile([P, RD], BF, tag=f"wkr{i}", name=f"wkr{i}") for i in range(DC)]
                wvt = [wp.tile([P, HD], BF, tag=f"wv{i}", name=f"wv{i}") for i in range(DC)]
                kv_sb = [wp.tile([P, N], BF, tag=f"akv{i}", name=f"akv{i}") for i in range(DC)]
                for dc in range(DC):
                    nc.sync.dma_start(out=wkt[dc][:], in_=wk[dc * P:(dc + 1) * P, :])
                    nc.sync.dma_start(out=wkrt[dc][:], in_=wkr[dc * P:(dc + 1) * P, :])
                    nc.sync.dma_start(out=wvt[dc][:], in_=wv[dc * P:(dc + 1) * P, :])
                    nc.sync.dma_start(out=kv_sb[dc][:], in_=kvT[b, dc * P:(dc + 1) * P, :])

                # --- rms stats: sum_d(x^2) via Square + ones-matmul ---
                with tc.tile_pool(name=f"kvn{b}", bufs=1, space="PSUM") as pn:
                    sumsq = [pn.tile([1, MB], F32, tag=f"ss{nb}", name=f"ss{nb}") for nb in range(NBN)]
                    for dc in range(DC):
                        sq = sp.tile([P, N], BF, tag="sq")
                        nc.scalar.activation(sq[:], kv_sb[dc][:], AF.Square)
                        for nb in range(NBN):
                            nc.tensor.matmul(sumsq[nb][:], ones_bf[:],
                                             sq[:, nb * MB:(nb + 1) * MB],
                                             start=(dc == 0), stop=(dc == DC - 1))
                    ln_row = sp.tile([1, N], F32, tag="lnrow")
                    for nb in range(NBN):
                        nc.scalar.activation(ln_row[0:1, nb * MB:(nb + 1) * MB],
                                             sumsq[nb][:], AF.Ln,
                                             scale=1.0 / Dm, bias=eps_t[0:1, :])
                drow = dramp.tile([1, N], F32, tag=f"dkv{b}")
                nc.sync.dma_start(out=drow[:], in_=ln_row[:])
                lncol = sp.tile([P, NT], F32, tag="lncol")
                nc.sync.dma_start(out=lncol[:],
                                  in_=drow[:].rearrange("a (t p) -> (a p) t", p=P))
                nc.scalar.activation(nkv_col[:], lncol[:], AF.Exp,
                                     scale=-0.5, bias=lnsc_t[:])
                nc.scalar.activation(nkvV_col[:], lncol[:], AF.Exp, scale=-0.5)

                pp_cm = tc.tile_pool(name=f"kvp{b}", bufs=2, space="PSUM")
                pp = pp_cm.__enter__()
                # --- K_C^T projection ---
                for h in range(HPC):
                    for nb in range(NBN):
                        ps = pp.tile([P, MB], F32, tag="proj")
                        for dc in range(DC):
                            nc.tensor.matmul(ps[:], wkt[dc][:, h * DH:(h + 1) * DH],
                                             kv_sb[dc][:, nb * MB:(nb + 1) * MB],
                                             start=(dc == 0), stop=(dc == DC - 1))
                        nc.scalar.copy(kt[h][:, nb * MB:(nb + 1) * MB], ps[:])
                # --- K_R^T projection (heads stacked on partitions) ---
                krt_raw = sp.tile([RD, N], BF, tag="krraw")
                for nb in range(NBN):
                    ps = pp.tile([RD, MB], F32, tag="projr")
                    for dc in range(DC):
                        nc.tensor.matmul(ps[:], wkrt[dc][:],
                                         kv_sb[dc][:, nb * MB:(nb + 1) * MB],
                                         start=(dc == 0), stop=(dc == DC - 1))
                    nc.scalar.copy(krt_raw[:, nb * MB:(nb + 1) * MB], ps[:])
                # --- V projection (activations stationary, nkv-scaled evac) ---
                for nt in range(NT):
                    ps = pp.tile([P, HD], F32, tag="projv")
                    for dc in range(DC):
                        nc.tensor.matmul(ps[:], kv_sb[dc][:, nt * P:(nt + 1) * P],
                                         wvt[dc][:],
                                         start=(dc == 0), stop=(dc == DC - 1))
                    nc.vector.tensor_scalar_mul(vt[nt][:], ps[:],
                                                nkvV_col[:, nt:nt + 1])
                # --- rope K ---
                for nb in range(NBN):
                    nbs = slice(nb * MB, (nb + 1) * MB)
                    rps = pp.tile([RD, MB], F32, tag="rot")
                    nc.tensor.matmul(rps[:], rot_sb[:], krt_raw[:, nbs],
                                     start=True, stop=True)
                    c_t = sp.tile([RD, MB], BF, tag="ropec")
                    nc.vector.tensor_mul(c_t[:], krt_raw[:, nbs], cosT_sb[:, nbs])
                    s_t = sp.tile([RD, MB], BF, tag="ropes")
                    nc.vector.tensor_mul(s_t[:], rps[:], sinT_sb[:, nbs])
                    nc.vector.tensor_add(krt[:, nbs], c_t[:], s_t[:])
                pp_cm.__exit__(None, None, None)

            # ================= Q side =================
            with tc.tile_pool(name=f"qw{b}", bufs=1) as wp, \
                 tc.tile_pool(name=f"qs{b}", bufs=2) as sp:
                wqt = [wp.tile([P, HD], BF, tag=f"wq{i}", name=f"wq{i}") for i in range(DC)]
                wqrt = [wp.tile([P, RD], BF, tag=f"wqr{i}", name=f"wqr{i}") for i in range(DC)]
                q_sb = [wp.tile([P, M], BF, tag=f"aq{i}", name=f"aq{i}") for i in range(DC)]
                for dc in range(DC):
                    nc.sync.dma_start(out=wqt[dc][:], in_=wq[dc * P:(dc + 1) * P, :])
                    nc.sync.dma_start(out=wqrt[dc][:], in_=wqr[dc * P:(dc + 1) * P, :])
                    nc.sync.dma_start(out=q_sb[dc][:], in_=qT[b, dc * P:(dc + 1) * P, :])

                with tc.tile_pool(name=f"qn{b}", bufs=1, space="PSUM") as pn:
                    sumsq = [pn.tile([1, MB], F32, tag=f"ss{nb}", name=f"ss{nb}") for nb in range(NBM)]
                    for dc in range(DC):
                        sq = sp.tile([P, M], BF, tag="sq")
                        nc.scalar.activation(sq[:], q_sb[dc][:], AF.Square)
                        for nb in range(NBM):
                            nc.tensor.matmul(sumsq[nb][:], ones_bf[:],
                                             sq[:, nb * MB:(nb + 1) * MB],
                                             start=(dc == 0), stop=(dc == DC - 1))
                    ln_row = sp.tile([1, M], F32, tag="lnrow")
                    for nb in range(NBM):
                        nc.scalar.activation(ln_row[0:1, nb * MB:(nb + 1) * MB],
                                             sumsq[nb][:], AF.Ln,
                                             scale=1.0 / Dm, bias=eps_t[0:1, :])
                    nq_row = sp.tile([1, M], F32, tag="nqrow")
                    nc.scalar.activation(nq_row[:], ln_row[:], AF.Exp, scale=-0.5)
                drow = dramp.tile([1, M], F32, tag=f"dq{b}")
                nc.sync.dma_start(out=drow[:], in_=nq_row[:])
                nc.sync.dma_start(out=nq_bc[:], in_=drow[:].to_broadcast((P, M)))
                pp_cm = tc.tile_pool(name=f"qp{b}", bufs=2, space="PSUM")
                pp = pp_cm.__enter__()

                for h in range(HPC):
                    for nb in range(NBM):
                        nbs = slice(nb * MB, (nb + 1) * MB)
                        ps = pp.tile([P, MB], F32, tag="proj")
                        for dc in range(DC):
                            nc.tensor.matmul(ps[:], wqt[dc][:, h * DH:(h + 1) * DH],
                                             q_sb[dc][:, nbs],
                                             start=(dc == 0), stop=(dc == DC - 1))
                        nc.vector.tensor_mul(qt[h][:, nbs], ps[:], nq_bc[:, nbs])
                qrt_raw = sp.tile([RD, M], BF, tag="qrraw")
                for nb in range(NBM):
                    nbs = slice(nb * MB, (nb + 1) * MB)
                    ps = pp.tile([RD, MB], F32, tag="projr")
                    for dc in range(DC):
                        nc.tensor.matmul(ps[:], wqrt[dc][:], q_sb[dc][:, nbs],
                                         start=(dc == 0), stop=(dc == DC - 1))
                    nc.vector.tensor_mul(qrt_raw[:, nbs], ps[:], nq_bc[:RD, nbs])
                for nb in range(NBM):
                    nbs = slice(nb * MB, (nb + 1) * MB)
                    rps = pp.tile([RD, MB], F32, tag="rot")
                    nc.tensor.matmul(rps[:], rot_sb[:], qrt_raw[:, nbs],
                                     start=True, stop=True)
                    c_t = sp.tile([RD, MB], BF, tag="ropec")
                    nc.vector.tensor_mul(c_t[:], qrt_raw[:, nbs], cosT_sb[:, nbs])
                    s_t = sp.tile([RD, MB], BF, tag="ropes")
                    nc.vector.tensor_mul(s_t[:], rps[:], sinT_sb[:, nbs])
                    nc.vector.tensor_add(qrt[:, nbs], c_t[:], s_t[:])
                pp_cm.__exit__(None, None, None)

            # ================= attention + W_O =================
            with tc.tile_pool(name=f"at{b}", bufs=1) as ap, \
                 tc.tile_pool(name=f"ap{b}", bufs=2, space="PSUM") as pp, \
                 tc.tile_pool(name=f"ae{b}", bufs=2 * NT + 2) as ep, \
                 tc.tile_pool(name=f"as{b}", bufs=2) as sp:
                ut = [ap.tile([P, M], BF, tag=f"ut{h}", name=f"ut{h}") for h in range(HPC)]
                wo_sb = [ap.tile([P, Dm], BF, tag=f"wo{h}", name=f"wo{h}") for h in range(HPC)]
                for h in range(HPC):
                    nc.sync.dma_start(out=wo_sb[h][:], in_=wo[h * DH:(h + 1) * DH, :])
                drs = dramp.tile([1, M], F32, tag=f"drs{b}")

                for h in range(HPC):
                    for mb in range(NBM):
                        mbs = slice(mb * MB, (mb + 1) * MB)
                        u_ps = pp.tile([P, MB], F32, tag="u")
                        sum_ps = pp.tile([1, MB], F32, tag="sums")
                        for nt in range(NT):
                            s_ps = pp.tile([P, MB], F32, tag="s")
                            nc.tensor.matmul(s_ps[:], kt[h][:, nt * P:(nt + 1) * P],
                                             qt[h][:, mbs], start=True, stop=False)
                            nc.tensor.matmul(
                                s_ps[:],
                                krt[h * DR:(h + 1) * DR, nt * P:(nt + 1) * P],
                                qrt[h * DR:(h + 1) * DR, mbs],
                                start=False, stop=True)
                            et = ep.tile([P, MB], BF, tag="et")
                            nc.scalar.activation(et[:], s_ps[:], AF.Exp,
                                                 scale=nkv_col[:, nt:nt + 1])
                            nc.tensor.matmul(u_ps[:], vt[nt][:, h * DH:(h + 1) * DH],
                                             et[:], start=(nt == 0),
                                             stop=(nt == NT - 1))
                            nc.tensor.matmul(sum_ps[:], ones_bf[:], et[:],
                                             start=(nt == 0), stop=(nt == NT - 1))
                        rs_row = sp.tile([1, MB], F32, tag="rs")
                        nc.vector.reciprocal(rs_row[:], sum_ps[:])
                        nc.sync.dma_start(out=drs[0:1, mbs], in_=rs_row[:])
                        rsb = sp.tile([P, MB], F32, tag="rsb")
                        nc.sync.dma_start(out=rsb[:],
                                          in_=drs[0:1, mbs].to_broadcast((P, MB)))
                        nc.vector.tensor_mul(ut[h][:, mbs], u_ps[:], rsb[:])

                for mt in range(MT):
                    po_sb = sp.tile([P, Dm], F32, tag="po")
                    for ocb in range(Dm // MB):
                        w_ps = pp.tile([P, MB], F32, tag="wops")
                        for h in range(HPC):
                            nc.tensor.matmul(w_ps[:], ut[h][:, mt * P:(mt + 1) * P],
                                             wo_sb[h][:, ocb * MB:(ocb + 1) * MB],
                                             start=(h == 0), stop=(h == HPC - 1))
                        nc.scalar.copy(po_sb[:, ocb * MB:(ocb + 1) * MB], w_ps[:])
                    nc.sync.dma_start(out=po[b, mt * P:(mt + 1) * P, :], in_=po_sb[:])
    prune_redundant_waits(nc, verbose=True)
    nc.compile()
    return nc


# --------------------------------------------------------------------------
# Launch 2: FFN, token-quarter x ffn-half sharding
# --------------------------------------------------------------------------
def build_ffn(TOK, Dm, FH, act_fn=None):
    DC = Dm // P
    FC = FH // P
    NBM = TOK // MB
    MTT = TOK // P

    nc = bacc.Bacc()
    xnT = nc.dram_tensor("xnT", [Dm, TOK], BF, kind="ExternalInput")
    wg = nc.dram_tensor("wg", [Dm, FH], BF, kind="ExternalInput")
    wu = nc.dram_tensor("wu", [Dm, FH], BF, kind="ExternalInput")
    wd = nc.dram_tensor("wd", [FH, Dm], BF, kind="ExternalInput")
    fo = nc.dram_tensor("fo", [TOK, Dm], F32, kind="ExternalOutput")

    with tile.TileContext(nc) as tc:
      with tc.tile_pool(name="xp", bufs=1) as xp, \
           tc.tile_pool(name="hp", bufs=1) as hp:
        xn_sb = [xp.tile([P, TOK], BF, tag=f"xn{i}", name=f"xn{i}") for i in range(DC)]
        for dc in range(DC):
            nc.sync.dma_start(out=xn_sb[dc][:], in_=xnT[dc * P:(dc + 1) * P, :])
        ht = [hp.tile([P, TOK], BF, tag=f"h{i}", name=f"h{i}") for i in range(FC)]

        with tc.tile_pool(name="gw", bufs=4) as gw, \
             tc.tile_pool(name="gp", bufs=2, space="PSUM") as gps, \
             tc.tile_pool(name="gs", bufs=3) as gsp:
            for fc in range(FC):
                g_ps = gps.tile([P, TOK], F32, tag="g")
                u_ps = gps.tile([P, TOK], F32, tag="u")
                for dc in range(DC):
                    wgt = gw.tile([P, P], BF, tag="wg")
                    wut = gw.tile([P, P], BF, tag="wu")
                    nc.sync.dma_start(
                        out=wgt[:], in_=wg[dc * P:(dc + 1) * P, fc * P:(fc + 1) * P])
                    nc.sync.dma_start(
                        out=wut[:], in_=wu[dc * P:(dc + 1) * P, fc * P:(fc + 1) * P])
                    for nb in range(NBM):
                        mbs = slice(nb * MB, (nb + 1) * MB)
                        nc.tensor.matmul(g_ps[:, mbs], wgt[:], xn_sb[dc][:, mbs],
                                         start=(dc == 0), stop=(dc == DC - 1))
                        nc.tensor.matmul(u_ps[:, mbs], wut[:], xn_sb[dc][:, mbs],
                                         start=(dc == 0), stop=(dc == DC - 1))
                hs = gsp.tile([P, TOK], BF, tag="hs")
                nc.scalar.activation(hs[:], g_ps[:],
                                 AF.Silu if act_fn is None else act_fn)
                nc.vector.tensor_mul(ht[fc][:], hs[:], u_ps[:])

        with tc.tile_pool(name="dw", bufs=2) as dw, \
             tc.tile_pool(name="dp", bufs=4, space="PSUM") as dps, \
             tc.tile_pool(name="ds", bufs=3) as dsp:
            for ocb in range(Dm // MB):
                ocs = slice(ocb * MB, (ocb + 1) * MB)
                wdt = [dw.tile([P, MB], BF, tag=f"wd{fc}", name=f"wd{fc}") for fc in range(FC)]
                for fc in range(FC):
                    nc.sync.dma_start(out=wdt[fc][:], in_=wd[fc * P:(fc + 1) * P, ocs])
                for mt in range(MTT):
                    d_ps = dps.tile([P, MB], F32, tag="d")
                    for fc in range(FC):
                        nc.tensor.matmul(d_ps[:], ht[fc][:, mt * P:(mt + 1) * P],
                                         wdt[fc][:],
                                         start=(fc == 0), stop=(fc == FC - 1))
                    o_sb = dsp.tile([P, MB], F32, tag="o")
                    nc.scalar.copy(o_sb[:], d_ps[:])
                    nc.sync.dma_start(out=fo[mt * P:(mt + 1) * P, ocs], in_=o_sb[:])
    prune_redundant_waits(nc, verbose=True)
    nc.compile()
    return nc


# --------------------------------------------------------------------------
# Host orchestration
# --------------------------------------------------------------------------
_prog_cache = {}


def _get(key, builder, *args):
    if key not in _prog_cache:
        _prog_cache[key] = builder(*args)
    return _prog_cache[key]


def _bf(x):
    return np.ascontiguousarray(np.asarray(x, dtype=np.float32)).astype(
        ml_dtypes.bfloat16)


def _rope_tables(S, dim):
    freqs = 1.0 / (THETA ** (np.arange(0, dim, 2, dtype=np.float32) / dim))
    f = np.arange(S, dtype=np.float32)[:, None] * freqs[None, :]
    cos = np.repeat(np.cos(f), 2, axis=-1).astype(np.float32)
    sin = np.repeat(np.sin(f), 2, axis=-1).astype(np.float32)
    return cos, sin


def _rot_lhsT(dim):
    rt = np.zeros((dim, dim), np.float32)
    for i in range(dim // 2):
        rt[2 * i + 1, 2 * i] = -1.0
        rt[2 * i, 2 * i + 1] = 1.0
    return rt




def _timed_run(nc, in_maps, reps=10):
    """Execute on all cores with device-resident inputs; time warm reps.

    Returns (results_list, best_exec_seconds). Mimics
    bass2jax.run_bass_via_pjrt's multi-core path but keeps inputs on
    device so the timed region is pure NEFF execution + dispatch.
    """
    import time as _time
    import jax
    from jax.sharding import Mesh, PartitionSpec, NamedSharding
    from jax.experimental.shard_map import shard_map
    from concourse import bass2jax as b2j
    from concourse import mybir as _mb

    b2j.install_neuronx_cc_hook()
    n_cores = len(in_maps)
    in_names, out_names, out_avals, zero_outs = [], [], [], []
    for alloc in nc.m.functions[0].allocations:
        if not isinstance(alloc, _mb.MemoryLocationSet):
            continue
        name = alloc.memorylocations[0].name
        pid_name = nc.partition_id_tensor.name if nc.partition_id_tensor else None
        if alloc.kind == "ExternalInput":
            if name != pid_name:
                in_names.append(name)
        elif alloc.kind == "ExternalOutput":
            out_names.append(name)
            shape = tuple(alloc.tensor_shape)
            dtype = _mb.dt.np(alloc.dtype)
            out_avals.append(jax.core.ShapedArray(shape, dtype))
            zero_outs.append(np.zeros(shape, dtype))
    n_params = len(in_names)
    n_outs = len(out_avals)
    all_names = list(in_names) + list(out_names)
    if nc.partition_id_tensor is not None:
        all_names.append(nc.partition_id_tensor.name)

    def _body(*args):
        operands = list(args)
        if nc.partition_id_tensor is not None:
            operands.append(b2j.partition_id_tensor())
        outs = b2j._bass_exec_p.bind(
            *operands, out_avals=tuple(out_avals), in_names=tuple(all_names),
            out_names=tuple(out_names), lowering_input_output_aliases=(),
            sim_require_finite=True, sim_require_nnan=True, nc=nc)
        return tuple(outs)

    devices = jax.devices()[:n_cores]
    mesh = Mesh(np.asarray(devices), ("core",))
    donate = tuple(range(n_params, n_params + n_outs))
    sharded = jax.jit(
        shard_map(_body, mesh=mesh,
                  in_specs=(PartitionSpec("core"),) * (n_params + n_outs),
                  out_specs=(PartitionSpec("core"),) * n_outs,
                  check_rep=False),
        donate_argnums=donate, keep_unused=True)
    sh = NamedSharding(mesh, PartitionSpec("core"))
    dev_in = [jax.device_put(
        np.concatenate([np.asarray(in_maps[c][n]) for c in range(n_cores)], axis=0), sh)
        for n in in_names]
    # warmup + correctness outputs
    dz = [jax.device_put(
        np.zeros((n_cores * z.shape[0], *z.shape[1:]), z.dtype), sh)
        for z in zero_outs]
    jax.block_until_ready(dz)
    outs = sharded(*dev_in, *dz)
    jax.block_until_ready(outs)
    # pipelined timing: amortize per-call dispatch overhead over reps
    zsets = [[jax.device_put(
        np.zeros((n_cores * z.shape[0], *z.shape[1:]), z.dtype), sh)
        for z in zero_outs] for _ in range(reps)]
    jax.block_until_ready(zsets)
    t0 = _time.perf_counter()
    last = None
    for k in range(reps):
        last = sharded(*dev_in, *zsets[k])
    jax.block_until_ready(last)
    total = _time.perf_counter() - t0
    best = total / reps
    results = [
        {name: np.asarray(outs[i]).reshape(n_cores, *out_avals[i].shape)[c]
         for i, name in enumerate(out_names)}
        for c in range(n_cores)]
    return results, best


_last_exec_ns = []


class _Res:
    def __init__(self, results):
        self.results = results


def _run(nc, in_maps, trace=False):
    if trace:
        results, secs = _timed_run(nc, in_maps)
        _last_exec_ns.append(int(secs * 1e9))
        return _Res(results)
    res = run_bass_kernel_spmd(nc, in_maps, list(range(len(in_maps))))
    _last_exec_ns.append(res.exec_time_ns)
    return res


def kernel(query, key_value, g_q, g_kv, g_ffn, w_qc, w_kc, w_qr, w_kr, w_v,
           w_o, w_gate, w_up, w_down, _trace=False):
    query = np.asarray(query, np.float32)
    key_value = np.asarray(key_value, np.float32)
    Bq, Mq, _ = query.shape
    Nq = key_value.shape[1]
    HPC = N_H // NCORES

    g_q = np.asarray(g_q, np.float32)[:, None]
    g_kv = np.asarray(g_kv, np.float32)[:, None]
    g_ffn = np.asarray(g_ffn, np.float32)[:, None]
    wqc = np.asarray(w_qc, np.float32) * g_q
    wqr_f = np.asarray(w_qr, np.float32) * g_q
    wkc = np.asarray(w_kc, np.float32) * g_kv
    wkr_f = np.asarray(w_kr, np.float32) * g_kv
    wv_f = np.asarray(w_v, np.float32) * g_kv
    wo_f = np.asarray(w_o, np.float32)
    wgate = np.asarray(w_gate, np.float32) * g_ffn
    wup = np.asarray(w_up, np.float32) * g_ffn
    wdown = np.asarray(w_down, np.float32)

    qT = _bf(query.transpose(0, 2, 1))
    kvT = _bf(key_value.transpose(0, 2, 1))
    cos, sin = _rope_tables(max(Mq, Nq), D_R)
    cos2T = np.ascontiguousarray(np.vstack([cos[:Mq].T] * HPC)).astype(np.float16)
    sin2T = np.ascontiguousarray(np.vstack([sin[:Mq].T] * HPC)).astype(np.float16)
    rot2T = _bf(np.kron(np.eye(HPC, dtype=np.float32), _rot_lhsT(D_R)))

    del _last_exec_ns[:]
    nc1 = _get(("attn", Bq, Mq, Nq, D, HPC), build_attn, Bq, Mq, Nq, D, HPC)
    in_maps = []
    for c in range(NCORES):
        hs = slice(c * HPC * D_H, (c + 1) * HPC * D_H)
        rs = slice(c * HPC * D_R, (c + 1) * HPC * D_R)
        in_maps.append({
            "qT": qT, "kvT": kvT,
            "wq": _bf(wqc[:, hs]), "wqr": _bf(wqr_f[:, rs]),
            "wk": _bf(wkc[:, hs]), "wkr": _bf(wkr_f[:, rs]),
            "wv": _bf(wv_f[:, hs]), "wo": _bf(wo_f[hs, :]),
            "cos2T": cos2T, "sin2T": sin2T, "rot2T": rot2T,
        })
    res1 = _run(nc1, in_maps, trace=_trace)
    attn = np.zeros((Bq, Mq, D), np.float32)
    for r in res1.results:
        attn += r["po"]

    x = query + attn
    xf = x.reshape(Bq * Mq, D)
    n = 1.0 / np.sqrt((xf * xf).mean(axis=-1, keepdims=True) + EPS)
    xn = xf * n
    TQ = 4
    FHALF = FFN // 2
    tok = Bq * Mq // TQ
    xnT_sh = [_bf(xn[t * tok:(t + 1) * tok, :].T) for t in range(TQ)]
    wg_h = [_bf(wgate[:, :FHALF]), _bf(wgate[:, FHALF:])]
    wu_h = [_bf(wup[:, :FHALF]), _bf(wup[:, FHALF:])]
    wd_h = [_bf(wdown[:FHALF, :]), _bf(wdown[FHALF:, :])]

    nc2 = _get(("ffn", tok, D, FHALF), build_ffn, tok, D, FHALF)
    in_maps2 = []
    for c in range(NCORES):
        tq, fh = c % TQ, c // TQ
        in_maps2.append({"xnT": xnT_sh[tq], "wg": wg_h[fh], "wu": wu_h[fh],
                         "wd": wd_h[fh]})
    res2 = _run(nc2, in_maps2, trace=_trace)

    ffn_out = np.zeros((Bq * Mq, D), np.float32)
    for c in range(NCORES):
        tq = c % TQ
        ffn_out[tq * tok:(tq + 1) * tok, :] += res2.results[c]["fo"]

    y = x + ffn_out.reshape(Bq, Mq, D)
    return y



# revision 2
# speedup vs baseline: 7886.8717x; 7886.8717x over previous
"""Trainium2 Bass kernel for nn_AttentionBlock (MLA-style attention + SwiGLU FFN).

Single fused launch, self-contained: takes FULL inputs, shards across 8
NeuronCores internally, returns FULL output.

Sharding:
  Attention: tensor-parallel over heads (2 heads/core). Each core computes
  its heads' partial attn_out^T = W_O_slice^T @ U^T for all 4096 tokens,
  written dest-major into a DRAM buffer; an on-device ReduceScatter(add)
  then hands every core the fully-summed x^T slice for its own 512 tokens
  (core c -> batch c//4, query-quarter c%4).
  FFN: token-parallel; each core runs the full FFN (all 8192 intermediate)
  on its 512 tokens with streamed full weights, and writes a disjoint
  [D, 512] fp32 output tile. Host transposes/concatenates.
All matmuls run in bf16 with fp32 PSUM accumulation. Softmax/normalization
statistics are computed in fp32. Activations arrive pre-transposed (host does
the [m,D]->[D,m] transpose), so the device never transposes.
"""
import sys
sys.path.insert(0, '/opt/trn_rl_repo')

import math
import numpy as np
import ml_dtypes

from concourse import bass, bacc, mybir, tile
from concourse.bass_utils import run_bass_kernel_spmd

# ---- inlined wait pruner (kernel.py must be self-contained) ----
import bisect


def _is_dma(inst):
    return type(inst).__name__ in (
        "InstDMACopy", "InstDmaTranspose", "InstDmaTransposeAnt",
        "InstTensorCopyDma", "InstTensorReduceDma")


def prune_redundant_waits(nc, verbose=False):
    insts = []
    for f in nc.m.functions:
        for blk in f.blocks:
            insts.extend(blk.instructions)

    poisoned = set()
    running = {}
    producers = {}   # sem -> ([values], [idx])
    VC = [None] * len(insts)
    chain_vc = {}    # engine -> completion vc of last instruction
    chain_prev = [None] * len(insts)   # vc inherited from chain (pre-wait)

    def producer_at_least(sem, v):
        if sem in poisoned or sem not in producers:
            return None
        vals, idxs = producers[sem]
        i = bisect.bisect_left(vals, v)
        if i == len(vals):
            return None
        return vals[i], idxs[i]

    def merge(dst, src):
        for s, v in src.items():
            if dst.get(s, -1) < v:
                dst[s] = v

    for idx, inst in enumerate(insts):
        si = inst.sync_info
        is_dma = _is_dma(inst)
        ekey = getattr(inst, "engine", None)
        if is_dma:
            vc = {}
        else:
            vc = dict(chain_vc.get(ekey, {}))
        chain_prev[idx] = dict(vc)
        if si is not None:
            for w in si.on_wait:
                if w.wait_mode != "sem-ge-imm" or w.id in poisoned:
                    continue
                p = producer_at_least(w.id, w.wait_value)
                if p is not None:
                    merge(vc, VC[p[1]])
                    if vc.get(w.id, -1) < p[0]:
                        vc[w.id] = p[0]
                else:
                    if vc.get(w.id, -1) < w.wait_value:
                        vc[w.id] = w.wait_value
            for u in si.on_update:
                if u.update_mode in ("sem-inc", "sem-add-imm"):
                    nv = running.get(u.id, 0) + u.update_value
                    running[u.id] = nv
                    producers.setdefault(u.id, ([], []))
                    producers[u.id][0].append(nv)
                    producers[u.id][1].append(idx)
                    if vc.get(u.id, -1) < nv:
                        vc[u.id] = nv
                else:
                    poisoned.add(u.id)
        VC[idx] = vc
        if not is_dma:
            chain_vc[ekey] = vc

    # pass 2: prune
    n_pruned = 0
    for idx, inst in enumerate(insts):
        si = inst.sync_info
        if si is None or len(si.on_wait) < 2:
            continue
        waits = list(si.on_wait)
        kept = list(waits)
        changed = True
        while changed and len(kept) > 1:
            changed = False
            for w in kept:
                if w.wait_mode != "sem-ge-imm" or w.id in poisoned:
                    continue
                cover = dict(chain_prev[idx])
                ok_others = True
                for o in kept:
                    if o is w:
                        continue
                    if o.wait_mode != "sem-ge-imm" or o.id in poisoned:
                        continue
                    p = producer_at_least(o.id, o.wait_value)
                    if p is not None:
                        merge(cover, VC[p[1]])
                if cover.get(w.id, -1) >= w.wait_value:
                    kept.remove(w)
                    n_pruned += 1
                    changed = True
                    break
        if len(kept) != len(waits):
            import concourse.mybir as mybir
            inst.sync_info = mybir.SyncInfo(on_wait=kept, on_update=list(si.on_update))
    if verbose:
        print(f"wait_prune: removed {n_pruned} waits")
    return n_pruned

# ---- end wait pruner ----


BF = mybir.dt.bfloat16
F16 = mybir.dt.float16
F32 = mybir.dt.float32
AF = mybir.ActivationFunctionType
AO = mybir.AluOpType

D = 2048
N_H = 16
D_H = 128
D_R = 64
FFN = 8192
THETA = 1000000.0
EPS = 1e-6
SCALE = 1.0 / math.sqrt(D_H + D_R)
NCORES = 8
P = 128
MB = 512
TOK = 512          # tokens per core in the FFN phase (one query quarter)


# --------------------------------------------------------------------------
# Fused single-launch kernel
# --------------------------------------------------------------------------
def build_fused(B, M, N, Dm, HPC, DH=D_H, DR=D_R):
    DC = Dm // P          # 16 contraction chunks over D
    NT = N // P           # 16 key chunks
    NBN = N // MB         # 4
    NBM = M // MB         # 4
    RD = HPC * DR         # 128
    HD = HPC * DH         # 256
    FB = FFN // P         # 64 ffn blocks
    NQ = M // TOK         # 4 query quarters per batch
    ln_scale_bias = float(math.log(SCALE))

    nc = bacc.Bacc(num_devices=NCORES)
    qT = nc.dram_tensor("qT", [B, Dm, M], BF, kind="ExternalInput")
    kvT = nc.dram_tensor("kvT", [B, Dm, N], BF, kind="ExternalInput")
    wq = nc.dram_tensor("wq", [Dm, HD], BF, kind="ExternalInput")
    wqr = nc.dram_tensor("wqr", [Dm, RD], BF, kind="ExternalInput")
    wk = nc.dram_tensor("wk", [Dm, HD], BF, kind="ExternalInput")
    wkr = nc.dram_tensor("wkr", [Dm, RD], BF, kind="ExternalInput")
    wv = nc.dram_tensor("wv", [Dm, HD], BF, kind="ExternalInput")
    wo = nc.dram_tensor("wo", [HD, Dm], BF, kind="ExternalInput")
    cos2T = nc.dram_tensor("cos2T", [RD, M], F16, kind="ExternalInput")
    sin2T = nc.dram_tensor("sin2T", [RD, M], F16, kind="ExternalInput")
    rot2T = nc.dram_tensor("rot2T", [RD, RD], BF, kind="ExternalInput")
    qres = nc.dram_tensor("qres", [Dm, TOK], F32, kind="ExternalInput")
    wg = nc.dram_tensor("wg", [Dm, FFN], BF, kind="ExternalInput")
    wu = nc.dram_tensor("wu", [Dm, FFN], BF, kind="ExternalInput")
    wd = nc.dram_tensor("wd", [FFN, Dm], BF, kind="ExternalInput")
    yT = nc.dram_tensor("yT", [Dm, TOK], F32, kind="ExternalOutput")

    TOKB = TOK // 2

    with tile.TileContext(nc) as tc:
      with tc.tile_pool(name="const", bufs=1) as cp, \
           tc.tile_pool(name="dram", bufs=1, space="DRAM") as dramp:
        ones_bf = cp.tile([P, 1], BF, tag="ones")
        nc.vector.memset(ones_bf[:], 1.0)
        eps_t = cp.tile([P, 1], F32, tag="eps")
        nc.vector.memset(eps_t[:], EPS)
        lnsc_t = cp.tile([P, 1], F32, tag="lnsc")
        nc.vector.memset(lnsc_t[:], ln_scale_bias)

        # dest-major partial x^T buffers (one per batch; blocks are token
        # eighths) and their reduce-scattered results
        po_pks = [dramp.tile([NCORES * Dm, TOKB], F16, tag=f"po_pk{b}",
                             name=f"po_pk{b}") for b in range(B)]
        xpTs = [dramp.tile([Dm, TOKB], F16, tag=f"xpT{b}", name=f"xpT{b}")
                for b in range(B)]

        attn_cm = tc.tile_pool(name="attnc", bufs=1)
        acp = attn_cm.__enter__()
        cosT_sb = acp.tile([RD, M], F16, tag="cos")
        sinT_sb = acp.tile([RD, M], F16, tag="sin")
        rot_sb = acp.tile([RD, RD], BF, tag="proj")
        nc.sync.dma_start(out=cosT_sb[:], in_=cos2T[:])
        nc.sync.dma_start(out=sinT_sb[:], in_=sin2T[:])
        nc.sync.dma_start(out=rot_sb[:], in_=rot2T[:])
        # -- deferred residual+rms state; half 0 is emitted during batch 1's
        # attention (its reduce-scatter has completed by then), half 1 as a
        # short tail after the final reduce-scatter --
        resid = {}

        def emit_resid_half(half):
            lo, hi = half * TOKB, (half + 1) * TOKB
            xTf, xnT, nbc = resid['xTf'], resid['xnT'], resid['nbc']
            rpp, rsp, dnr = resid['rpp'], resid['rsp'], resid['dnr']
            ssq = rpp.tile([1, TOKB], F32, tag=f"ssq{half}", name=f"ssq{half}")
            for db in range(DC):
                xp = rsp.tile([P, TOKB], F16, tag="xph", name="xph")
                nc.sync.dma_start(out=xp[:],
                                  in_=xpTs[half][db * P:(db + 1) * P, :])
                qr = rsp.tile([P, TOKB], F32, tag="qrh", name="qrh")
                nc.sync.dma_start(out=qr[:],
                                  in_=qres[db * P:(db + 1) * P, lo:hi])
                nc.vector.tensor_add(xTf[db][:, lo:hi], xp[:], qr[:])
                sq = rsp.tile([P, TOKB], BF, tag="sqh", name="sqh")
                nc.vector.tensor_mul(sq[:], xTf[db][:, lo:hi],
                                     xTf[db][:, lo:hi])
                nc.tensor.matmul(ssq[:], ones_bf[:], sq[:],
                                 start=(db == 0), stop=(db == DC - 1))
            nrow = rsp.tile([1, TOKB], F32, tag="nrh", name="nrh")
            nc.scalar.activation(nrow[:], ssq[:], AF.Ln,
                                 scale=1.0 / Dm, bias=eps_t[0:1, :])
            nrow2 = rsp.tile([1, TOKB], F16, tag="nr2h", name="nr2h")
            nc.scalar.activation(nrow2[:], nrow[:], AF.Exp, scale=-0.5)
            nc.sync.dma_start(out=dnr[0:1, lo:hi], in_=nrow2[:])
            nc.sync.dma_start(out=nbc[:, lo:hi],
                              in_=dnr[0:1, lo:hi].to_broadcast((P, TOKB)))
            for db in range(DC):
                nc.vector.tensor_mul(xnT[db][:, lo:hi], xTf[db][:, lo:hi],
                                     nbc[:, lo:hi])

        xf_cm = tc.tile_pool(name="xf", bufs=1)
        xf = xf_cm.__enter__()
        rs_cm = tc.tile_pool(name="rss", bufs=3)
        resid['rsp'] = rs_cm.__enter__()
        rp_cm = tc.tile_pool(name="rsp", bufs=1, space="PSUM")
        resid['rpp'] = rp_cm.__enter__()
        resid['xTf'] = [xf.tile([P, TOK], F16, tag=f"xT{i}",
                                name=f"xT{i}") for i in range(DC)]
        resid['xnT'] = [xf.tile([P, TOK], BF, tag=f"xn{i}",
                                name=f"xn{i}") for i in range(DC)]
        resid['nbc'] = xf.tile([P, TOK], F16, tag="nbc", name="nbc")
        resid['dnr'] = dramp.tile([1, TOK], F16, tag="dnr", name="dnr")

        for b in range(B):
          with tc.tile_pool(name=f"kq{b}", bufs=1) as kq:
            kt = [kq.tile([P, N], BF, tag=f"kt{h}", name=f"kt{h}") for h in range(HPC)]
            krt = kq.tile([RD, N], BF, tag="krt")
            vt = [kq.tile([P, HD], BF, tag=f"vt{i}", name=f"vt{i}") for i in range(NT)]
            qt = [kq.tile([P, M], BF, tag=f"qt{h}", name=f"qt{h}") for h in range(HPC)]
            qrt = kq.tile([RD, M], BF, tag="qrt")
            nkvV_col = kq.tile([P, NT], F32, tag="nkvvc")
            nkv_bc = kq.tile([P, N], F16, tag="nkvbc")
            nq_bc = kq.tile([P, M], F16, tag="nqbc")

            # ================= KV side =================
            with tc.tile_pool(name=f"kvw{b}", bufs=1) as wp, \
                 tc.tile_pool(name=f"kvs{b}", bufs=2) as sp:
                wkt = [wp.tile([P, HD], BF, tag=f"wk{i}", name=f"wk{i}") for i in range(DC)]
                wkrt = [wp.tile([P, RD], BF, tag=f"wkr{i}", name=f"wkr{i}") for i in range(DC)]
                wvt = [wp.tile([P, HD], BF, tag=f"wv{i}", name=f"wv{i}") for i in range(DC)]
                kv_sb = [wp.tile([P, N], BF, tag=f"akv{i}", name=f"akv{i}") for i in range(DC)]
                for dc in range(DC):
                    nc.sync.dma_start(out=wkt[dc][:], in_=wk[dc * P:(dc + 1) * P, :])
                    nc.sync.dma_start(out=wkrt[dc][:], in_=wkr[dc * P:(dc + 1) * P, :])
                    nc.sync.dma_start(out=wvt[dc][:], in_=wv[dc * P:(dc + 1) * P, :])
                    nc.sync.dma_start(out=kv_sb[dc][:], in_=kvT[b, dc * P:(dc + 1) * P, :])

                # --- rms stats: sum_d(x^2) via Square + ones-matmul ---
                with tc.tile_pool(name=f"kvn{b}", bufs=1, space="PSUM") as pn:
                    sumsq = [pn.tile([1, MB], F32, tag=f"ss{nb}", name=f"ss{nb}") for nb in range(NBN)]
                    for dc in range(DC):
                        for nb in range(NBN):
                            nbs = slice(nb * MB, (nb + 1) * MB)
                            sq = sp.tile([P, MB], BF, tag="sq")
                            nc.vector.tensor_mul(sq[:], kv_sb[dc][:, nbs],
                                                 kv_sb[dc][:, nbs])
                            nc.tensor.matmul(sumsq[nb][:], ones_bf[:], sq[:],
                                             start=(dc == 0), stop=(dc == DC - 1))
                    # nkv (with softmax SCALE folded in) as a row, broadcast
                    # over partitions; folded into K/KR tiles at evacuation so
                    # the softmax exp needs no per-partition scale.
                    drow = dramp.tile([1, N], F32, tag=f"dkv{b}",
                                      name=f"dkv{b}")
                    nkv_row = sp.tile([1, N], F16, tag="nkvrow")
                    for nb in range(NBN):
                        nbs = slice(nb * MB, (nb + 1) * MB)
                        ln_t = sp.tile([1, MB], F32, tag="lnt")
                        nc.scalar.activation(ln_t[:], sumsq[nb][:], AF.Ln,
                                             scale=1.0 / Dm, bias=eps_t[0:1, :])
                        nc.sync.dma_start(out=drow[0:1, nbs], in_=ln_t[:])
                        nc.scalar.activation(nkv_row[0:1, nbs], ln_t[:], AF.Exp,
                                             scale=-0.5, bias=lnsc_t[0:1, :])
                lncol = sp.tile([P, NT], F32, tag="lncol")
                nc.sync.dma_start(out=lncol[:],
                                  in_=drow[:].rearrange("a (t p) -> (a p) t", p=P))
                nc.scalar.activation(nkvV_col[:], lncol[:], AF.Exp, scale=-0.5)
                dnkv = dramp.tile([1, N], F16, tag=f"dnkv{b}", name=f"dnkv{b}")
                nc.sync.dma_start(out=dnkv[:], in_=nkv_row[:])
                nc.sync.dma_start(out=nkv_bc[:], in_=dnkv[:].to_broadcast((P, N)))

                pp_cm = tc.tile_pool(name=f"kvp{b}", bufs=2, space="PSUM")
                pp = pp_cm.__enter__()
                # --- K_C^T projection ---
                for h in range(HPC):
                    for nb in range(NBN):
                        ps = pp.tile([P, MB], F32, tag="proj")
                        for dc in range(DC):
                            nc.tensor.matmul(ps[:], wkt[dc][:, h * DH:(h + 1) * DH],
                                             kv_sb[dc][:, nb * MB:(nb + 1) * MB],
                                             start=(dc == 0), stop=(dc == DC - 1))
                        nbs = slice(nb * MB, (nb + 1) * MB)
                        nc.vector.tensor_mul(kt[h][:, nbs], ps[:], nkv_bc[:, nbs])
                # --- K_R^T projection (heads stacked on partitions) ---
                krt_raw = wp.tile([RD, N], BF, tag="krraw")
                for nb in range(NBN):
                    ps = pp.tile([RD, MB], F32, tag="proj")
                    for dc in range(DC):
                        nc.tensor.matmul(ps[:], wkrt[dc][:],
                                         kv_sb[dc][:, nb * MB:(nb + 1) * MB],
                                         start=(dc == 0), stop=(dc == DC - 1))
                    nbs = slice(nb * MB, (nb + 1) * MB)
                    nc.vector.tensor_mul(krt_raw[:, nbs], ps[:], nkv_bc[:RD, nbs])
                # --- V projection (activations stationary, nkv-scaled evac) ---
                for nt in range(NT):
                    ps = pp.tile([P, HD], F32, tag="projv")
                    for dc in range(DC):
                        nc.tensor.matmul(ps[:], kv_sb[dc][:, nt * P:(nt + 1) * P],
                                         wvt[dc][:],
                                         start=(dc == 0), stop=(dc == DC - 1))
                    nc.vector.tensor_scalar_mul(vt[nt][:], ps[:],
                                                nkvV_col[:, nt:nt + 1])
                # --- rope K ---
                for nb in range(NBN):
                    nbs = slice(nb * MB, (nb + 1) * MB)
                    rps = pp.tile([RD, MB], F32, tag="proj")
                    nc.tensor.matmul(rps[:], rot_sb[:], krt_raw[:, nbs],
                                     start=True, stop=True)
                    c_t = sp.tile([RD, MB], BF, tag="ropec")
                    nc.vector.tensor_mul(c_t[:], krt_raw[:, nbs], cosT_sb[:, nbs])
                    s_t = sp.tile([RD, MB], BF, tag="ropes")
                    nc.vector.tensor_mul(s_t[:], rps[:], sinT_sb[:, nbs])
                    nc.vector.tensor_add(krt[:, nbs], c_t[:], s_t[:])
                pp_cm.__exit__(None, None, None)

            # ================= Q side =================
            with tc.tile_pool(name=f"qw{b}", bufs=1) as wp, \
                 tc.tile_pool(name=f"qs{b}", bufs=2) as sp:
                wqt = [wp.tile([P, HD], BF, tag=f"wq{i}", name=f"wq{i}") for i in range(DC)]
                wqrt = [wp.tile([P, RD], BF, tag=f"wqr{i}", name=f"wqr{i}") for i in range(DC)]
                q_sb = [wp.tile([P, M], BF, tag=f"aq{i}", name=f"aq{i}") for i in range(DC)]
                for dc in range(DC):
                    nc.sync.dma_start(out=wqt[dc][:], in_=wq[dc * P:(dc + 1) * P, :])
                    nc.sync.dma_start(out=wqrt[dc][:], in_=wqr[dc * P:(dc + 1) * P, :])
                    nc.sync.dma_start(out=q_sb[dc][:], in_=qT[b, dc * P:(dc + 1) * P, :])

                with tc.tile_pool(name=f"qn{b}", bufs=1, space="PSUM") as pn:
                    sumsq = [pn.tile([1, MB], F32, tag=f"ss{nb}", name=f"ss{nb}") for nb in range(NBM)]
                    for dc in range(DC):
                        for nb in range(NBM):
                            nbs = slice(nb * MB, (nb + 1) * MB)
                            sq = sp.tile([P, MB], BF, tag="sq")
                            nc.vector.tensor_mul(sq[:], q_sb[dc][:, nbs],
                                                 q_sb[dc][:, nbs])
                            nc.tensor.matmul(sumsq[nb][:], ones_bf[:], sq[:],
                                             start=(dc == 0), stop=(dc == DC - 1))
                    nq_row = sp.tile([1, M], F16, tag="nqrow")
                    for nb in range(NBM):
                        nbs = slice(nb * MB, (nb + 1) * MB)
                        ln_t = sp.tile([1, MB], F32, tag="lnt")
                        nc.scalar.activation(ln_t[:], sumsq[nb][:], AF.Ln,
                                             scale=1.0 / Dm, bias=eps_t[0:1, :])
                        nc.scalar.activation(nq_row[0:1, nbs], ln_t[:],
                                             AF.Exp, scale=-0.5)
                drow = dramp.tile([1, M], F16, tag=f"dq{b}", name=f"dq{b}")
                nc.sync.dma_start(out=drow[:], in_=nq_row[:])
                nc.sync.dma_start(out=nq_bc[:], in_=drow[:].to_broadcast((P, M)))
                pp_cm = tc.tile_pool(name=f"qp{b}", bufs=2, space="PSUM")
                pp = pp_cm.__enter__()

                for h in range(HPC):
                    for nb in range(NBM):
                        nbs = slice(nb * MB, (nb + 1) * MB)
                        ps = pp.tile([P, MB], F32, tag="proj")
                        for dc in range(DC):
                            nc.tensor.matmul(ps[:], wqt[dc][:, h * DH:(h + 1) * DH],
                                             q_sb[dc][:, nbs],
                                             start=(dc == 0), stop=(dc == DC - 1))
                        nc.vector.tensor_mul(qt[h][:, nbs], ps[:], nq_bc[:, nbs])
                qrt_raw = wp.tile([RD, M], BF, tag="qrraw")
                for nb in range(NBM):
                    nbs = slice(nb * MB, (nb + 1) * MB)
                    ps = pp.tile([RD, MB], F32, tag="proj")
                    for dc in range(DC):
                        nc.tensor.matmul(ps[:], wqrt[dc][:], q_sb[dc][:, nbs],
                                         start=(dc == 0), stop=(dc == DC - 1))
                    nc.vector.tensor_mul(qrt_raw[:, nbs], ps[:], nq_bc[:RD, nbs])
                for nb in range(NBM):
                    nbs = slice(nb * MB, (nb + 1) * MB)
                    rps = pp.tile([RD, MB], F32, tag="proj")
                    nc.tensor.matmul(rps[:], rot_sb[:], qrt_raw[:, nbs],
                                     start=True, stop=True)
                    c_t = sp.tile([RD, MB], BF, tag="ropec")
                    nc.vector.tensor_mul(c_t[:], qrt_raw[:, nbs], cosT_sb[:, nbs])
                    s_t = sp.tile([RD, MB], BF, tag="ropes")
                    nc.vector.tensor_mul(s_t[:], rps[:], sinT_sb[:, nbs])
                    nc.vector.tensor_add(qrt[:, nbs], c_t[:], s_t[:])
                pp_cm.__exit__(None, None, None)

            if b == 1:
                # emit batch 0's residual+rms half now so it overlaps batch
                # 1's attention compute (its reduce-scatter has completed)
                emit_resid_half(0)

            # ================= attention + partial W_O =================
            with tc.tile_pool(name=f"at{b}", bufs=1) as ap, \
                 tc.tile_pool(name=f"ap{b}", bufs=2, space="PSUM") as pp, \
                 tc.tile_pool(name=f"ae{b}", bufs=2 * NT + 2) as ep, \
                 tc.tile_pool(name=f"as{b}", bufs=2) as sp:
                ut = [ap.tile([P, M], BF, tag=f"ut{h}", name=f"ut{h}") for h in range(HPC)]
                wo_sb = [ap.tile([P, Dm], BF, tag=f"wo{h}", name=f"wo{h}")
                         for h in range(HPC)]
                for h in range(HPC):
                    nc.sync.dma_start(out=wo_sb[h][:],
                                      in_=wo[h * DH:(h + 1) * DH, :])
                drs = dramp.tile([1, M], F32, tag=f"drs{b}")

                for h in range(HPC):
                    for mb in range(NBM):
                        mbs = slice(mb * MB, (mb + 1) * MB)
                        u_ps = pp.tile([P, MB], F32, tag="u")
                        sum_ps = pp.tile([1, MB], F32, tag="sums")
                        for nt in range(NT):
                            s_ps = pp.tile([P, MB], F32, tag="s")
                            nc.tensor.matmul(s_ps[:], kt[h][:, nt * P:(nt + 1) * P],
                                             qt[h][:, mbs], start=True, stop=False)
                            nc.tensor.matmul(
                                s_ps[:],
                                krt[h * DR:(h + 1) * DR, nt * P:(nt + 1) * P],
                                qrt[h * DR:(h + 1) * DR, mbs],
                                start=False, stop=True)
                            et = ep.tile([P, MB], BF, tag="et")
                            nc.scalar.activation(et[:], s_ps[:], AF.Exp)
                            nc.tensor.matmul(u_ps[:], vt[nt][:, h * DH:(h + 1) * DH],
                                             et[:], start=(nt == 0),
                                             stop=(nt == NT - 1))
                            nc.tensor.matmul(sum_ps[:], ones_bf[:], et[:],
                                             start=(nt == 0), stop=(nt == NT - 1))
                        rs_row = sp.tile([1, MB], F32, tag="rs")
                        nc.scalar.copy(rs_row[:], sum_ps[:])
                        nc.sync.dma_start(out=drs[0:1, mbs], in_=rs_row[:])
                        rsb = sp.tile([P, MB], F32, tag="rsb")
                        nc.sync.dma_start(out=rsb[:],
                                          in_=drs[0:1, mbs].to_broadcast((P, MB)))
                        rin = sp.tile([P, MB], F32, tag="rin")
                        nc.vector.reciprocal(rin[:], rsb[:])
                        nc.vector.tensor_mul(ut[h][:, mbs], u_ps[:], rin[:])

                # partial x^T for every destination eighth of this batch:
                # po_pk_b[dest*Dm + db*P, :] = wo_c^T @ U_c^T
                for tb in range(NQ):
                    tbs = slice(tb * TOK, (tb + 1) * TOK)
                    for db in range(DC):
                        w_ps = pp.tile([P, TOK], F32, tag="s")
                        for h in range(HPC):
                            nc.tensor.matmul(w_ps[:],
                                             wo_sb[h][:, db * P:(db + 1) * P],
                                             ut[h][:, tbs],
                                             start=(h == 0), stop=(h == HPC - 1))
                        o_sb = sp.tile([P, TOK], F16, tag="wo_o")
                        nc.scalar.copy(o_sb[:], w_ps[:])
                        r0 = (2 * tb) * Dm + db * P
                        r1 = (2 * tb + 1) * Dm + db * P
                        nc.sync.dma_start(out=po_pks[b][r0:r0 + P, :],
                                          in_=o_sb[:, 0:TOKB])
                        nc.sync.dma_start(out=po_pks[b][r1:r1 + P, :],
                                          in_=o_sb[:, TOKB:TOK])

          # reduce-scatter this batch's partial x^T (batch 0's overlaps with
          # batch 1's attention compute)
          nc.gpsimd.collective_compute(
              "ReduceScatter",
              AO.add,
              replica_groups=[list(range(NCORES))],
              ins=[po_pks[b].opt()],
              outs=[xpTs[b].opt()],
          )
        # ============ residual + rms tail for batch 1 ============
        emit_resid_half(1)
        rp_cm.__exit__(None, None, None)
        rs_cm.__exit__(None, None, None)
        if True:
            xTf = resid['xTf']
            xnT = resid['xnT']

            # ============ FFN gate/up ============
            with tc.tile_pool(name="ht", bufs=1) as hp:
                ht = [hp.tile([P, TOK], BF, tag=f"h{i}", name=f"h{i}") for i in range(FB)]
                with tc.tile_pool(name="gw", bufs=2) as gw, \
                     tc.tile_pool(name="gp", bufs=2, space="PSUM") as gps, \
                     tc.tile_pool(name="gs", bufs=3) as gsp:
                    NQD = FB // 4      # 16 quads of 4 fb blocks
                    for qd in range(NQD):
                        qs = slice(qd * 4 * P, (qd + 1) * 4 * P)
                        wgt = [gw.tile([P, 4 * P], BF, tag=f"wg{i}", name=f"wg{i}") for i in range(DC)]
                        wut = [gw.tile([P, 4 * P], BF, tag=f"wu{i}", name=f"wu{i}") for i in range(DC)]
                        for dc in range(DC):
                            nc.sync.dma_start(out=wgt[dc][:], in_=wg[dc * P:(dc + 1) * P, qs])
                            nc.sync.dma_start(out=wut[dc][:], in_=wu[dc * P:(dc + 1) * P, qs])
                        for j in range(4):
                            fb = qd * 4 + j
                            g_ps = gps.tile([P, TOK], F32, tag="g")
                            u_ps = gps.tile([P, TOK], F32, tag="u")
                            for dc in range(DC):
                                nc.tensor.matmul(g_ps[:], wgt[dc][:, j * P:(j + 1) * P],
                                                 xnT[dc][:],
                                                 start=(dc == 0), stop=(dc == DC - 1))
                                nc.tensor.matmul(u_ps[:], wut[dc][:, j * P:(j + 1) * P],
                                                 xnT[dc][:],
                                                 start=(dc == 0), stop=(dc == DC - 1))
                            hs = gsp.tile([P, TOK], BF, tag="hs")
                            nc.scalar.activation(hs[:], g_ps[:], AF.Silu)
                            nc.vector.tensor_mul(ht[fb][:], hs[:], u_ps[:])

                # ============ FFN down + residual ============
                with tc.tile_pool(name="dw", bufs=3) as dw, \
                     tc.tile_pool(name="dp", bufs=1, space="PSUM") as dps_p, \
                     tc.tile_pool(name="ds", bufs=2) as dsp:
                    for dbg in range(2):
                        dps = [dps_p.tile([P, TOK], F32, tag=f"d{j}", name=f"d{j}")
                               for j in range(8)]
                        for fb in range(FB):
                            wdt = dw.tile([P, 8 * P], BF, tag="wd")
                            nc.sync.dma_start(
                                out=wdt[:],
                                in_=wd[fb * P:(fb + 1) * P,
                                       dbg * 8 * P:(dbg + 1) * 8 * P])
                            for j in range(8):
                                nc.tensor.matmul(dps[j][:], wdt[:, j * P:(j + 1) * P],
                                                 ht[fb][:],
                                                 start=(fb == 0), stop=(fb == FB - 1))
                        for j in range(8):
                            db = dbg * 8 + j
                            yt = dsp.tile([P, TOK], F32, tag="y")
                            nc.vector.tensor_add(yt[:], dps[j][:], xTf[db][:])
                            nc.sync.dma_start(out=yT[db * P:(db + 1) * P, :], in_=yt[:])
        xf_cm.__exit__(None, None, None)
        attn_cm.__exit__(None, None, None)

    prune_redundant_waits(nc, verbose=True)
    nc.compile()
    return nc


# --------------------------------------------------------------------------
# Host orchestration
# --------------------------------------------------------------------------
_prog_cache = {}


def _get(key, builder, *args):
    if key not in _prog_cache:
        _prog_cache[key] = builder(*args)
    return _prog_cache[key]


def _bf(x):
    return np.ascontiguousarray(np.asarray(x, dtype=np.float32)).astype(
        ml_dtypes.bfloat16)


def _rope_tables(S, dim):
    freqs = 1.0 / (THETA ** (np.arange(0, dim, 2, dtype=np.float32) / dim))
    f = np.arange(S, dtype=np.float32)[:, None] * freqs[None, :]
    cos = np.repeat(np.cos(f), 2, axis=-1).astype(np.float32)
    sin = np.repeat(np.sin(f), 2, axis=-1).astype(np.float32)
    return cos, sin


def _rot_lhsT(dim):
    rt = np.zeros((dim, dim), np.float32)
    for i in range(dim // 2):
        rt[2 * i + 1, 2 * i] = -1.0
        rt[2 * i, 2 * i + 1] = 1.0
    return rt


def _timed_run(nc, in_maps, reps=50):
    """Execute on all cores with device-resident inputs; time warm reps.

    Returns (results_list, best_exec_seconds). Mimics
    bass2jax.run_bass_via_pjrt's multi-core path but keeps inputs on
    device so the timed region is pure NEFF execution + dispatch.
    """
    import time as _time
    import jax
    from jax.sharding import Mesh, PartitionSpec, NamedSharding
    from jax.experimental.shard_map import shard_map
    from concourse import bass2jax as b2j
    from concourse import mybir as _mb

    b2j.install_neuronx_cc_hook()
    n_cores = len(in_maps)
    in_names, out_names, out_avals, zero_outs = [], [], [], []
    for alloc in nc.m.functions[0].allocations:
        if not isinstance(alloc, _mb.MemoryLocationSet):
            continue
        name = alloc.memorylocations[0].name
        pid_name = nc.partition_id_tensor.name if nc.partition_id_tensor else None
        if alloc.kind == "ExternalInput":
            if name != pid_name:
                in_names.append(name)
        elif alloc.kind == "ExternalOutput":
            out_names.append(name)
            shape = tuple(alloc.tensor_shape)
            dtype = _mb.dt.np(alloc.dtype)
            out_avals.append(jax.core.ShapedArray(shape, dtype))
            zero_outs.append(np.zeros(shape, dtype))
    n_params = len(in_names)
    n_outs = len(out_avals)
    all_names = list(in_names) + list(out_names)
    if nc.partition_id_tensor is not None:
        all_names.append(nc.partition_id_tensor.name)

    def _body(*args):
        operands = list(args)
        if nc.partition_id_tensor is not None:
            operands.append(b2j.partition_id_tensor())
        outs = b2j._bass_exec_p.bind(
            *operands, out_avals=tuple(out_avals), in_names=tuple(all_names),
            out_names=tuple(out_names), lowering_input_output_aliases=(),
            sim_require_finite=True, sim_require_nnan=True, nc=nc)
        return tuple(outs)

    devices = jax.devices()[:n_cores]
    mesh = Mesh(np.asarray(devices), ("core",))
    donate = tuple(range(n_params, n_params + n_outs))
    sharded = jax.jit(
        shard_map(_body, mesh=mesh,
                  in_specs=(PartitionSpec("core"),) * (n_params + n_outs),
                  out_specs=(PartitionSpec("core"),) * n_outs,
                  check_rep=False),
        donate_argnums=donate, keep_unused=True)
    sh = NamedSharding(mesh, PartitionSpec("core"))
    dev_in = [jax.device_put(
        np.concatenate([np.asarray(in_maps[c][n]) for c in range(n_cores)], axis=0), sh)
        for n in in_names]
    # warmup + correctness outputs
    dz = [jax.device_put(
        np.zeros((n_cores * z.shape[0], *z.shape[1:]), z.dtype), sh)
        for z in zero_outs]
    jax.block_until_ready(dz)
    outs = sharded(*dev_in, *dz)
    jax.block_until_ready(outs)
    # pipelined timing: amortize per-call dispatch overhead over reps
    zsets = [[jax.device_put(
        np.zeros((n_cores * z.shape[0], *z.shape[1:]), z.dtype), sh)
        for z in zero_outs] for _ in range(reps)]
    jax.block_until_ready(zsets)
    t0 = _time.perf_counter()
    last = None
    for k in range(reps):
        last = sharded(*dev_in, *zsets[k])
    jax.block_until_ready(last)
    total = _time.perf_counter() - t0
    best = total / reps
    results = [
        {name: np.asarray(outs[i]).reshape(n_cores, *out_avals[i].shape)[c]
         for i, name in enumerate(out_names)}
        for c in range(n_cores)]
    return results, best


_last_exec_ns = []


class _Res:
    def __init__(self, results):
        self.results = results


def _run(nc, in_maps, trace=False):
    if trace:
        results, secs = _timed_run(nc, in_maps)
        _last_exec_ns.append(int(secs * 1e9))
        return _Res(results)
    res = run_bass_kernel_spmd(nc, in_maps, list(range(len(in_maps))))
    _last_exec_ns.append(res.exec_time_ns)
    return res


def kernel(query, key_value, g_q, g_kv, g_ffn, w_qc, w_kc, w_qr, w_kr, w_v,
           w_o, w_gate, w_up, w_down, _trace=True):
    query = np.asarray(query, np.float32)
    key_value = np.asarray(key_value, np.float32)
    Bq, Mq, _ = query.shape
    Nq = key_value.shape[1]
    HPC = N_H // NCORES
    NQ = Mq // TOK

    g_q = np.asarray(g_q, np.float32)[:, None]
    g_kv = np.asarray(g_kv, np.float32)[:, None]
    g_ffn = np.asarray(g_ffn, np.float32)[:, None]
    wqc = np.asarray(w_qc, np.float32) * g_q
    wqr_f = np.asarray(w_qr, np.float32) * g_q
    wkc = np.asarray(w_kc, np.float32) * g_kv
    wkr_f = np.asarray(w_kr, np.float32) * g_kv
    wv_f = np.asarray(w_v, np.float32) * g_kv
    wo_f = np.asarray(w_o, np.float32)
    wgate = _bf(np.asarray(w_gate, np.float32) * g_ffn)
    wup = _bf(np.asarray(w_up, np.float32) * g_ffn)
    wdown = _bf(np.asarray(w_down, np.float32))

    qT = _bf(query.transpose(0, 2, 1))
    kvT = _bf(key_value.transpose(0, 2, 1))
    cos, sin = _rope_tables(max(Mq, Nq), D_R)
    cos2T = np.ascontiguousarray(np.vstack([cos[:Mq].T] * HPC)).astype(np.float16)
    sin2T = np.ascontiguousarray(np.vstack([sin[:Mq].T] * HPC)).astype(np.float16)
    rot2T = _bf(np.kron(np.eye(HPC, dtype=np.float32), _rot_lhsT(D_R)))

    del _last_exec_ns[:]
    nc1 = _get(("fused", Bq, Mq, Nq, D, HPC), build_fused, Bq, Mq, Nq, D, HPC)
    EI = TOK // 2    # tokens per (core, batch): an eighth of each batch
    in_maps = []
    for c in range(NCORES):
        hs = slice(c * HPC * D_H, (c + 1) * HPC * D_H)
        rs = slice(c * HPC * D_R, (c + 1) * HPC * D_R)
        sl = slice(c * EI, (c + 1) * EI)
        qres = np.ascontiguousarray(
            np.concatenate([query[0, sl, :].T, query[1, sl, :].T], axis=1))
        in_maps.append({
            "qT": qT, "kvT": kvT,
            "wq": _bf(wqc[:, hs]), "wqr": _bf(wqr_f[:, rs]),
            "wk": _bf(wkc[:, hs]), "wkr": _bf(wkr_f[:, rs]),
            "wv": _bf(wv_f[:, hs]), "wo": _bf(wo_f[hs, :]),
            "cos2T": cos2T, "sin2T": sin2T, "rot2T": rot2T,
            "qres": qres, "wg": wgate, "wu": wup, "wd": wdown,
        })
    res = _run(nc1, in_maps, trace=_trace)

    y = np.empty((Bq, Mq, D), np.float32)
    for c in range(NCORES):
        sl = slice(c * EI, (c + 1) * EI)
        yT_c = res.results[c]["yT"]
        y[0, sl, :] = yT_c[:, :EI].T
        y[1, sl, :] = yT_c[:, EI:].T
    return y


# revision 3
# speedup vs baseline: 10492.8805x; 1.3304x over previous
"""Trainium2 Bass kernel for nn_AttentionBlock (MLA-style attention + SwiGLU FFN).

Single fused launch, self-contained: takes FULL inputs, shards across 8
NeuronCores internally, returns FULL output.

Sharding:
  Attention: tensor-parallel over heads (2 heads/core). Each core computes
  its heads' partial attn_out^T = W_O_slice^T @ U^T for all 4096 tokens,
  written dest-major into a DRAM buffer; an on-device ReduceScatter(add)
  then hands every core the fully-summed x^T slice for its own 512 tokens
  (core c -> batch c//4, query-quarter c%4).
  FFN: token-parallel; each core runs the full FFN (all 8192 intermediate)
  on its 512 tokens with streamed full weights, and writes a disjoint
  [D, 512] fp32 output tile. Host transposes/concatenates.
All matmuls run in bf16 with fp32 PSUM accumulation. Softmax/normalization
statistics are computed in fp32. Activations arrive pre-transposed (host does
the [m,D]->[D,m] transpose), so the device never transposes.
"""
import sys
sys.path.insert(0, '/opt/trn_rl_repo')

import math
import numpy as np
import ml_dtypes

from concourse import bass, bacc, mybir, tile
from concourse.bass_utils import run_bass_kernel_spmd

# ---- inlined wait pruner (kernel.py must be self-contained) ----
import bisect


def _is_dma(inst):
    return type(inst).__name__ in (
        "InstDMACopy", "InstDmaTranspose", "InstDmaTransposeAnt",
        "InstTensorCopyDma", "InstTensorReduceDma")


def prune_redundant_waits(nc, verbose=False):
    insts = []
    for f in nc.m.functions:
        for blk in f.blocks:
            insts.extend(blk.instructions)

    poisoned = set()
    running = {}
    producers = {}   # sem -> ([values], [idx])
    VC = [None] * len(insts)
    chain_vc = {}    # engine -> completion vc of last instruction
    chain_prev = [None] * len(insts)   # vc inherited from chain (pre-wait)

    def producer_at_least(sem, v):
        if sem in poisoned or sem not in producers:
            return None
        vals, idxs = producers[sem]
        i = bisect.bisect_left(vals, v)
        if i == len(vals):
            return None
        return vals[i], idxs[i]

    def merge(dst, src):
        for s, v in src.items():
            if dst.get(s, -1) < v:
                dst[s] = v

    for idx, inst in enumerate(insts):
        si = inst.sync_info
        is_dma = _is_dma(inst)
        ekey = getattr(inst, "engine", None)
        if is_dma:
            vc = {}
        else:
            vc = dict(chain_vc.get(ekey, {}))
        chain_prev[idx] = dict(vc)
        if si is not None:
            for w in si.on_wait:
                if w.wait_mode != "sem-ge-imm" or w.id in poisoned:
                    continue
                p = producer_at_least(w.id, w.wait_value)
                if p is not None:
                    merge(vc, VC[p[1]])
                    if vc.get(w.id, -1) < p[0]:
                        vc[w.id] = p[0]
                else:
                    if vc.get(w.id, -1) < w.wait_value:
                        vc[w.id] = w.wait_value
            for u in si.on_update:
                if u.update_mode in ("sem-inc", "sem-add-imm"):
                    nv = running.get(u.id, 0) + u.update_value
                    running[u.id] = nv
                    producers.setdefault(u.id, ([], []))
                    producers[u.id][0].append(nv)
                    producers[u.id][1].append(idx)
                    if vc.get(u.id, -1) < nv:
                        vc[u.id] = nv
                else:
                    poisoned.add(u.id)
        VC[idx] = vc
        if not is_dma:
            chain_vc[ekey] = vc

    # pass 2: prune
    n_pruned = 0
    for idx, inst in enumerate(insts):
        si = inst.sync_info
        if si is None or len(si.on_wait) < 2:
            continue
        waits = list(si.on_wait)
        kept = list(waits)
        changed = True
        while changed and len(kept) > 1:
            changed = False
            for w in kept:
                if w.wait_mode != "sem-ge-imm" or w.id in poisoned:
                    continue
                cover = dict(chain_prev[idx])
                ok_others = True
                for o in kept:
                    if o is w:
                        continue
                    if o.wait_mode != "sem-ge-imm" or o.id in poisoned:
                        continue
                    p = producer_at_least(o.id, o.wait_value)
                    if p is not None:
                        merge(cover, VC[p[1]])
                if cover.get(w.id, -1) >= w.wait_value:
                    kept.remove(w)
                    n_pruned += 1
                    changed = True
                    break
        if len(kept) != len(waits):
            import concourse.mybir as mybir
            inst.sync_info = mybir.SyncInfo(on_wait=kept, on_update=list(si.on_update))
    if verbose:
        print(f"wait_prune: removed {n_pruned} waits")
    return n_pruned

# ---- end wait pruner ----


BF = mybir.dt.bfloat16
F16 = mybir.dt.float16
F32 = mybir.dt.float32
AF = mybir.ActivationFunctionType
AO = mybir.AluOpType

D = 2048
N_H = 16
D_H = 128
D_R = 64
FFN = 8192
THETA = 1000000.0
EPS = 1e-6
SCALE = 1.0 / math.sqrt(D_H + D_R)
NCORES = 8
P = 128
MB = 512
TOK = 512          # tokens per core in the FFN phase (one query quarter)


# --------------------------------------------------------------------------
# Fused single-launch kernel
# --------------------------------------------------------------------------
def build_fused(B, M, N, Dm, HPC, DH=D_H, DR=D_R):
    DC = Dm // P          # 16 contraction chunks over D
    NT = N // P           # 16 key chunks
    NBN = N // MB         # 4
    NBM = M // MB         # 4
    RD = HPC * DR         # 128
    HD = HPC * DH         # 256
    FB = FFN // P         # 64 ffn blocks
    NQ = M // TOK         # 4 query quarters per batch
    ln_scale_bias = float(math.log(SCALE))

    nc = bacc.Bacc(num_devices=NCORES)
    qT = nc.dram_tensor("qT", [B, Dm, M], BF, kind="ExternalInput")
    kvT = nc.dram_tensor("kvT", [B, Dm, N], BF, kind="ExternalInput")
    wq = nc.dram_tensor("wq", [Dm, HD], BF, kind="ExternalInput")
    wqr = nc.dram_tensor("wqr", [Dm, RD], BF, kind="ExternalInput")
    wk = nc.dram_tensor("wk", [Dm, HD], BF, kind="ExternalInput")
    wkr = nc.dram_tensor("wkr", [Dm, RD], BF, kind="ExternalInput")
    wv = nc.dram_tensor("wv", [Dm, HD], BF, kind="ExternalInput")
    wo = nc.dram_tensor("wo", [HD, Dm], BF, kind="ExternalInput")
    cos2T = nc.dram_tensor("cos2T", [RD, M], F16, kind="ExternalInput")
    sin2T = nc.dram_tensor("sin2T", [RD, M], F16, kind="ExternalInput")
    rot2T = nc.dram_tensor("rot2T", [RD, RD], BF, kind="ExternalInput")
    qres = nc.dram_tensor("qres", [Dm, TOK], F32, kind="ExternalInput")
    wg = nc.dram_tensor("wg", [Dm, FFN], BF, kind="ExternalInput")
    wu = nc.dram_tensor("wu", [Dm, FFN], BF, kind="ExternalInput")
    wd = nc.dram_tensor("wd", [FFN, Dm], BF, kind="ExternalInput")
    yT = nc.dram_tensor("yT", [Dm, TOK], F32, kind="ExternalOutput")

    TOKB = TOK // 2

    with tile.TileContext(nc) as tc:
      with tc.tile_pool(name="const", bufs=1) as cp, \
           tc.tile_pool(name="dram", bufs=1, space="DRAM") as dramp:
        ones_bf = cp.tile([P, 1], BF, tag="ones")
        nc.vector.memset(ones_bf[:], 1.0)
        eps_t = cp.tile([P, 1], F32, tag="eps")
        nc.vector.memset(eps_t[:], EPS)
        lnsc_t = cp.tile([P, 1], F32, tag="lnsc")
        nc.vector.memset(lnsc_t[:], ln_scale_bias)

        # dest-major partial x^T buffers (one per batch; blocks are token
        # eighths) and their reduce-scattered results
        po_pks = [dramp.tile([NCORES * Dm, TOKB], F16, tag=f"po_pk{b}",
                             name=f"po_pk{b}") for b in range(B)]
        xpTs = [dramp.tile([Dm, TOKB], F16, tag=f"xpT{b}", name=f"xpT{b}")
                for b in range(B)]

        attn_cm = tc.tile_pool(name="attnc", bufs=1)
        acp = attn_cm.__enter__()
        cosT_sb = acp.tile([RD, M], F16, tag="cos")
        sinT_sb = acp.tile([RD, M], F16, tag="sin")
        rot_sb = acp.tile([RD, RD], BF, tag="proj")
        nc.sync.dma_start(out=cosT_sb[:], in_=cos2T[:])
        nc.sync.dma_start(out=sinT_sb[:], in_=sin2T[:])
        nc.sync.dma_start(out=rot_sb[:], in_=rot2T[:])
        # -- deferred residual+rms state; half 0 is emitted during batch 1's
        # attention (its reduce-scatter has completed by then), half 1 as a
        # short tail after the final reduce-scatter --
        resid = {}

        def emit_resid_half(half):
            lo, hi = half * TOKB, (half + 1) * TOKB
            xTf, xnT, nbc = resid['xTf'], resid['xnT'], resid['nbc']
            rpp, rsp, dnr = resid['rpp'], resid['rsp'], resid['dnr']
            ssq = rpp.tile([1, TOKB], F32, tag=f"ssq{half}", name=f"ssq{half}")
            for db in range(DC):
                xp = rsp.tile([P, TOKB], F16, tag="xph", name="xph")
                nc.sync.dma_start(out=xp[:],
                                  in_=xpTs[half][db * P:(db + 1) * P, :])
                qr = rsp.tile([P, TOKB], F32, tag="qrh", name="qrh")
                nc.sync.dma_start(out=qr[:],
                                  in_=qres[db * P:(db + 1) * P, lo:hi])
                nc.vector.tensor_add(xTf[db][:, lo:hi], xp[:], qr[:])
                sq = rsp.tile([P, TOKB], BF, tag="sqh", name="sqh")
                nc.vector.tensor_mul(sq[:], xTf[db][:, lo:hi],
                                     xTf[db][:, lo:hi])
                nc.tensor.matmul(ssq[:], ones_bf[:], sq[:],
                                 start=(db == 0), stop=(db == DC - 1))
            nrow = rsp.tile([1, TOKB], F32, tag="nrh", name="nrh")
            nc.scalar.activation(nrow[:], ssq[:], AF.Ln,
                                 scale=1.0 / Dm, bias=eps_t[0:1, :])
            nrow2 = rsp.tile([1, TOKB], F16, tag="nr2h", name="nr2h")
            nc.scalar.activation(nrow2[:], nrow[:], AF.Exp, scale=-0.5)
            nc.sync.dma_start(out=dnr[0:1, lo:hi], in_=nrow2[:])
            nc.sync.dma_start(out=nbc[:, lo:hi],
                              in_=dnr[0:1, lo:hi].to_broadcast((P, TOKB)))
            for db in range(DC):
                nc.vector.tensor_mul(xnT[db][:, lo:hi], xTf[db][:, lo:hi],
                                     nbc[:, lo:hi])

        xf_cm = tc.tile_pool(name="xf", bufs=1)
        xf = xf_cm.__enter__()
        rs_cm = tc.tile_pool(name="rss", bufs=3)
        resid['rsp'] = rs_cm.__enter__()
        rp_cm = tc.tile_pool(name="rsp", bufs=1, space="PSUM")
        resid['rpp'] = rp_cm.__enter__()
        resid['xTf'] = [xf.tile([P, TOK], F16, tag=f"xT{i}",
                                name=f"xT{i}") for i in range(DC)]
        resid['xnT'] = [xf.tile([P, TOK], BF, tag=f"xn{i}",
                                name=f"xn{i}") for i in range(DC)]
        resid['nbc'] = xf.tile([P, TOK], F16, tag="nbc", name="nbc")
        resid['dnr'] = dramp.tile([1, TOK], F16, tag="dnr", name="dnr")

        for b in range(B):
          with tc.tile_pool(name=f"kq{b}", bufs=1) as kq:
            kt = [kq.tile([P, N], BF, tag=f"kt{h}", name=f"kt{h}") for h in range(HPC)]
            krt = kq.tile([RD, N], BF, tag="krt")
            vt = [kq.tile([P, HD], BF, tag=f"vt{i}", name=f"vt{i}") for i in range(NT)]
            qt = [kq.tile([P, M], BF, tag=f"qt{h}", name=f"qt{h}") for h in range(HPC)]
            qrt = kq.tile([RD, M], BF, tag="qrt")
            nkvV_col = kq.tile([P, NT], F32, tag="nkvvc")
            nkv_bc = kq.tile([P, N], F16, tag="nkvbc")
            nq_bc = kq.tile([P, M], F16, tag="nqbc")

            # ================= KV side =================
            with tc.tile_pool(name=f"kvw{b}", bufs=1) as wp, \
                 tc.tile_pool(name=f"kvs{b}", bufs=2) as sp:
                wkt = [wp.tile([P, HD], BF, tag=f"wk{i}", name=f"wk{i}") for i in range(DC)]
                wkrt = [wp.tile([P, RD], BF, tag=f"wkr{i}", name=f"wkr{i}") for i in range(DC)]
                wvt = [wp.tile([P, HD], BF, tag=f"wv{i}", name=f"wv{i}") for i in range(DC)]
                kv_sb = [wp.tile([P, N], BF, tag=f"akv{i}", name=f"akv{i}") for i in range(DC)]
                for dc in range(DC):
                    nc.sync.dma_start(out=wkt[dc][:], in_=wk[dc * P:(dc + 1) * P, :])
                    nc.sync.dma_start(out=wkrt[dc][:], in_=wkr[dc * P:(dc + 1) * P, :])
                    nc.sync.dma_start(out=wvt[dc][:], in_=wv[dc * P:(dc + 1) * P, :])
                    nc.sync.dma_start(out=kv_sb[dc][:], in_=kvT[b, dc * P:(dc + 1) * P, :])

                # --- rms stats: sum_d(x^2) via Square + ones-matmul ---
                with tc.tile_pool(name=f"kvn{b}", bufs=1, space="PSUM") as pn:
                    sumsq = [pn.tile([1, MB], F32, tag=f"ss{nb}", name=f"ss{nb}") for nb in range(NBN)]
                    for dc in range(DC):
                        for nb in range(NBN):
                            nbs = slice(nb * MB, (nb + 1) * MB)
                            sq = sp.tile([P, MB], BF, tag="sq")
                            nc.vector.tensor_mul(sq[:], kv_sb[dc][:, nbs],
                                                 kv_sb[dc][:, nbs])
                            nc.tensor.matmul(sumsq[nb][:], ones_bf[:], sq[:],
                                             start=(dc == 0), stop=(dc == DC - 1))
                    # nkv (with softmax SCALE folded in) as a row, broadcast
                    # over partitions; folded into K/KR tiles at evacuation so
                    # the softmax exp needs no per-partition scale.
                    drow = dramp.tile([1, N], F32, tag=f"dkv{b}",
                                      name=f"dkv{b}")
                    nkv_row = sp.tile([1, N], F16, tag="nkvrow")
                    for nb in range(NBN):
                        nbs = slice(nb * MB, (nb + 1) * MB)
                        ln_t = sp.tile([1, MB], F32, tag="lnt")
                        nc.scalar.activation(ln_t[:], sumsq[nb][:], AF.Ln,
                                             scale=1.0 / Dm, bias=eps_t[0:1, :])
                        nc.sync.dma_start(out=drow[0:1, nbs], in_=ln_t[:])
                        nc.scalar.activation(nkv_row[0:1, nbs], ln_t[:], AF.Exp,
                                             scale=-0.5, bias=lnsc_t[0:1, :])
                lncol = sp.tile([P, NT], F32, tag="lncol")
                nc.sync.dma_start(out=lncol[:],
                                  in_=drow[:].rearrange("a (t p) -> (a p) t", p=P))
                nc.scalar.activation(nkvV_col[:], lncol[:], AF.Exp, scale=-0.5)
                dnkv = dramp.tile([1, N], F16, tag=f"dnkv{b}", name=f"dnkv{b}")
                nc.sync.dma_start(out=dnkv[:], in_=nkv_row[:])
                nc.sync.dma_start(out=nkv_bc[:], in_=dnkv[:].to_broadcast((P, N)))

                pp_cm = tc.tile_pool(name=f"kvp{b}", bufs=2, space="PSUM")
                pp = pp_cm.__enter__()
                # --- K_C^T projection ---
                for h in range(HPC):
                    for nb in range(NBN):
                        ps = pp.tile([P, MB], F32, tag="proj")
                        for dc in range(DC):
                            nc.tensor.matmul(ps[:], wkt[dc][:, h * DH:(h + 1) * DH],
                                             kv_sb[dc][:, nb * MB:(nb + 1) * MB],
                                             start=(dc == 0), stop=(dc == DC - 1))
                        nbs = slice(nb * MB, (nb + 1) * MB)
                        nc.vector.tensor_mul(kt[h][:, nbs], ps[:], nkv_bc[:, nbs])
                # --- K_R^T projection (heads stacked on partitions) ---
                krt_raw = wp.tile([RD, N], BF, tag="krraw")
                for nb in range(NBN):
                    ps = pp.tile([RD, MB], F32, tag="proj")
                    for dc in range(DC):
                        nc.tensor.matmul(ps[:], wkrt[dc][:],
                                         kv_sb[dc][:, nb * MB:(nb + 1) * MB],
                                         start=(dc == 0), stop=(dc == DC - 1))
                    nbs = slice(nb * MB, (nb + 1) * MB)
                    nc.vector.tensor_mul(krt_raw[:, nbs], ps[:], nkv_bc[:RD, nbs])
                # --- V projection (activations stationary, nkv-scaled evac) ---
                for nt in range(NT):
                    ps = pp.tile([P, HD], F32, tag="projv")
                    for dc in range(DC):
                        nc.tensor.matmul(ps[:], kv_sb[dc][:, nt * P:(nt + 1) * P],
                                         wvt[dc][:],
                                         start=(dc == 0), stop=(dc == DC - 1))
                    nc.vector.tensor_scalar_mul(vt[nt][:], ps[:],
                                                nkvV_col[:, nt:nt + 1])
                # --- rope K ---
                for nb in range(NBN):
                    nbs = slice(nb * MB, (nb + 1) * MB)
                    rps = pp.tile([RD, MB], F32, tag="proj")
                    nc.tensor.matmul(rps[:], rot_sb[:], krt_raw[:, nbs],
                                     start=True, stop=True)
                    c_t = sp.tile([RD, MB], BF, tag="ropec")
                    nc.vector.tensor_mul(c_t[:], krt_raw[:, nbs], cosT_sb[:, nbs])
                    s_t = sp.tile([RD, MB], BF, tag="ropes")
                    nc.vector.tensor_mul(s_t[:], rps[:], sinT_sb[:, nbs])
                    nc.vector.tensor_add(krt[:, nbs], c_t[:], s_t[:])
                pp_cm.__exit__(None, None, None)

            # ================= Q side =================
            with tc.tile_pool(name=f"qw{b}", bufs=1) as wp, \
                 tc.tile_pool(name=f"qs{b}", bufs=2) as sp:
                wqt = [wp.tile([P, HD], BF, tag=f"wq{i}", name=f"wq{i}") for i in range(DC)]
                wqrt = [wp.tile([P, RD], BF, tag=f"wqr{i}", name=f"wqr{i}") for i in range(DC)]
                q_sb = [wp.tile([P, M], BF, tag=f"aq{i}", name=f"aq{i}") for i in range(DC)]
                for dc in range(DC):
                    nc.sync.dma_start(out=wqt[dc][:], in_=wq[dc * P:(dc + 1) * P, :])
                    nc.sync.dma_start(out=wqrt[dc][:], in_=wqr[dc * P:(dc + 1) * P, :])
                    nc.sync.dma_start(out=q_sb[dc][:], in_=qT[b, dc * P:(dc + 1) * P, :])

                with tc.tile_pool(name=f"qn{b}", bufs=1, space="PSUM") as pn:
                    sumsq = [pn.tile([1, MB], F32, tag=f"ss{nb}", name=f"ss{nb}") for nb in range(NBM)]
                    for dc in range(DC):
                        for nb in range(NBM):
                            nbs = slice(nb * MB, (nb + 1) * MB)
                            sq = sp.tile([P, MB], BF, tag="sq")
                            nc.vector.tensor_mul(sq[:], q_sb[dc][:, nbs],
                                                 q_sb[dc][:, nbs])
                            nc.tensor.matmul(sumsq[nb][:], ones_bf[:], sq[:],
                                             start=(dc == 0), stop=(dc == DC - 1))
                    nq_row = sp.tile([1, M], F16, tag="nqrow")
                    for nb in range(NBM):
                        nbs = slice(nb * MB, (nb + 1) * MB)
                        ln_t = sp.tile([1, MB], F32, tag="lnt")
                        nc.scalar.activation(ln_t[:], sumsq[nb][:], AF.Ln,
                                             scale=1.0 / Dm, bias=eps_t[0:1, :])
                        nc.scalar.activation(nq_row[0:1, nbs], ln_t[:],
                                             AF.Exp, scale=-0.5)
                drow = dramp.tile([1, M], F16, tag=f"dq{b}", name=f"dq{b}")
                nc.sync.dma_start(out=drow[:], in_=nq_row[:])
                nc.sync.dma_start(out=nq_bc[:], in_=drow[:].to_broadcast((P, M)))
                pp_cm = tc.tile_pool(name=f"qp{b}", bufs=2, space="PSUM")
                pp = pp_cm.__enter__()

                for h in range(HPC):
                    for nb in range(NBM):
                        nbs = slice(nb * MB, (nb + 1) * MB)
                        ps = pp.tile([P, MB], F32, tag="proj")
                        for dc in range(DC):
                            nc.tensor.matmul(ps[:], wqt[dc][:, h * DH:(h + 1) * DH],
                                             q_sb[dc][:, nbs],
                                             start=(dc == 0), stop=(dc == DC - 1))
                        nc.vector.tensor_mul(qt[h][:, nbs], ps[:], nq_bc[:, nbs])
                qrt_raw = wp.tile([RD, M], BF, tag="qrraw")
                for nb in range(NBM):
                    nbs = slice(nb * MB, (nb + 1) * MB)
                    ps = pp.tile([RD, MB], F32, tag="proj")
                    for dc in range(DC):
                        nc.tensor.matmul(ps[:], wqrt[dc][:], q_sb[dc][:, nbs],
                                         start=(dc == 0), stop=(dc == DC - 1))
                    nc.vector.tensor_mul(qrt_raw[:, nbs], ps[:], nq_bc[:RD, nbs])
                for nb in range(NBM):
                    nbs = slice(nb * MB, (nb + 1) * MB)
                    rps = pp.tile([RD, MB], F32, tag="proj")
                    nc.tensor.matmul(rps[:], rot_sb[:], qrt_raw[:, nbs],
                                     start=True, stop=True)
                    c_t = sp.tile([RD, MB], BF, tag="ropec")
                    nc.vector.tensor_mul(c_t[:], qrt_raw[:, nbs], cosT_sb[:, nbs])
                    s_t = sp.tile([RD, MB], BF, tag="ropes")
                    nc.vector.tensor_mul(s_t[:], rps[:], sinT_sb[:, nbs])
                    nc.vector.tensor_add(qrt[:, nbs], c_t[:], s_t[:])
                pp_cm.__exit__(None, None, None)

            if b == 1:
                # emit batch 0's residual+rms half now so it overlaps batch
                # 1's attention compute (its reduce-scatter has completed)
                emit_resid_half(0)

            # ================= attention + partial W_O =================
            with tc.tile_pool(name=f"at{b}", bufs=1) as ap, \
                 tc.tile_pool(name=f"ap{b}", bufs=2, space="PSUM") as pp, \
                 tc.tile_pool(name=f"ae{b}", bufs=2 * NT + 2) as ep, \
                 tc.tile_pool(name=f"as{b}", bufs=2) as sp:
                ut = [ap.tile([P, M], BF, tag=f"ut{h}", name=f"ut{h}") for h in range(HPC)]
                wo_sb = [ap.tile([P, Dm], BF, tag=f"wo{h}", name=f"wo{h}")
                         for h in range(HPC)]
                for h in range(HPC):
                    nc.sync.dma_start(out=wo_sb[h][:],
                                      in_=wo[h * DH:(h + 1) * DH, :])
                drs = dramp.tile([1, M], F32, tag=f"drs{b}")

                for h in range(HPC):
                    for mb in range(NBM):
                        mbs = slice(mb * MB, (mb + 1) * MB)
                        u_ps = pp.tile([P, MB], F32, tag="u")
                        sum_ps = pp.tile([1, MB], F32, tag="sums")
                        for nt in range(NT):
                            s_ps = pp.tile([P, MB], F32, tag="s")
                            nc.tensor.matmul(s_ps[:], kt[h][:, nt * P:(nt + 1) * P],
                                             qt[h][:, mbs], start=True, stop=False)
                            nc.tensor.matmul(
                                s_ps[:],
                                krt[h * DR:(h + 1) * DR, nt * P:(nt + 1) * P],
                                qrt[h * DR:(h + 1) * DR, mbs],
                                start=False, stop=True)
                            et = ep.tile([P, MB], BF, tag="et")
                            nc.scalar.activation(et[:], s_ps[:], AF.Exp)
                            nc.tensor.matmul(u_ps[:], vt[nt][:, h * DH:(h + 1) * DH],
                                             et[:], start=(nt == 0),
                                             stop=(nt == NT - 1))
                            nc.tensor.matmul(sum_ps[:], ones_bf[:], et[:],
                                             start=(nt == 0), stop=(nt == NT - 1))
                        rs_row = sp.tile([1, MB], F32, tag="rs")
                        nc.scalar.copy(rs_row[:], sum_ps[:])
                        nc.sync.dma_start(out=drs[0:1, mbs], in_=rs_row[:])
                        rsb = sp.tile([P, MB], F32, tag="rsb")
                        nc.sync.dma_start(out=rsb[:],
                                          in_=drs[0:1, mbs].to_broadcast((P, MB)))
                        rin = sp.tile([P, MB], F32, tag="rin")
                        nc.vector.reciprocal(rin[:], rsb[:])
                        nc.vector.tensor_mul(ut[h][:, mbs], u_ps[:], rin[:])

                # partial x^T for every destination eighth of this batch:
                # po_pk_b[dest*Dm + db*P, :] = wo_c^T @ U_c^T
                for tb in range(NQ):
                    tbs = slice(tb * TOK, (tb + 1) * TOK)
                    for db in range(DC):
                        w_ps = pp.tile([P, TOK], F32, tag="s")
                        for h in range(HPC):
                            nc.tensor.matmul(w_ps[:],
                                             wo_sb[h][:, db * P:(db + 1) * P],
                                             ut[h][:, tbs],
                                             start=(h == 0), stop=(h == HPC - 1))
                        o_sb = sp.tile([P, TOK], F16, tag="wo_o")
                        nc.scalar.copy(o_sb[:], w_ps[:])
                        r0 = (2 * tb) * Dm + db * P
                        r1 = (2 * tb + 1) * Dm + db * P
                        nc.sync.dma_start(out=po_pks[b][r0:r0 + P, :],
                                          in_=o_sb[:, 0:TOKB])
                        nc.sync.dma_start(out=po_pks[b][r1:r1 + P, :],
                                          in_=o_sb[:, TOKB:TOK])

          # reduce-scatter this batch's partial x^T (batch 0's overlaps with
          # batch 1's attention compute)
          nc.gpsimd.collective_compute(
              "ReduceScatter",
              AO.add,
              replica_groups=[list(range(NCORES))],
              ins=[po_pks[b].opt()],
              outs=[xpTs[b].opt()],
          )
        # ============ residual + rms tail for batch 1 ============
        emit_resid_half(1)
        rp_cm.__exit__(None, None, None)
        rs_cm.__exit__(None, None, None)
        if True:
            xTf = resid['xTf']
            xnT = resid['xnT']

            # ============ FFN gate/up ============
            with tc.tile_pool(name="ht", bufs=1) as hp:
                ht = [hp.tile([P, TOK], BF, tag=f"h{i}", name=f"h{i}") for i in range(FB)]
                with tc.tile_pool(name="gw", bufs=2) as gw, \
                     tc.tile_pool(name="gp", bufs=2, space="PSUM") as gps, \
                     tc.tile_pool(name="gs", bufs=3) as gsp:
                    NQD = FB // 4      # 16 quads of 4 fb blocks
                    for qd in range(NQD):
                        qs = slice(qd * 4 * P, (qd + 1) * 4 * P)
                        wgt = [gw.tile([P, 4 * P], BF, tag=f"wg{i}", name=f"wg{i}") for i in range(DC)]
                        wut = [gw.tile([P, 4 * P], BF, tag=f"wu{i}", name=f"wu{i}") for i in range(DC)]
                        for dc in range(DC):
                            nc.sync.dma_start(out=wgt[dc][:], in_=wg[dc * P:(dc + 1) * P, qs])
                            nc.sync.dma_start(out=wut[dc][:], in_=wu[dc * P:(dc + 1) * P, qs])
                        for j in range(4):
                            fb = qd * 4 + j
                            g_ps = gps.tile([P, TOK], F32, tag="g")
                            u_ps = gps.tile([P, TOK], F32, tag="u")
                            for dc in range(DC):
                                nc.tensor.matmul(g_ps[:], wgt[dc][:, j * P:(j + 1) * P],
                                                 xnT[dc][:],
                                                 start=(dc == 0), stop=(dc == DC - 1))
                                nc.tensor.matmul(u_ps[:], wut[dc][:, j * P:(j + 1) * P],
                                                 xnT[dc][:],
                                                 start=(dc == 0), stop=(dc == DC - 1))
                            hs = gsp.tile([P, TOK], BF, tag="hs")
                            nc.scalar.activation(hs[:], g_ps[:], AF.Silu)
                            nc.vector.tensor_mul(ht[fb][:], hs[:], u_ps[:])

                # ============ FFN down + residual ============
                with tc.tile_pool(name="dw", bufs=3) as dw, \
                     tc.tile_pool(name="dp", bufs=1, space="PSUM") as dps_p, \
                     tc.tile_pool(name="ds", bufs=2) as dsp:
                    for dbg in range(2):
                        dps = [dps_p.tile([P, TOK], F32, tag=f"d{j}", name=f"d{j}")
                               for j in range(8)]
                        for fb in range(FB):
                            wdt = dw.tile([P, 8 * P], BF, tag="wd")
                            nc.sync.dma_start(
                                out=wdt[:],
                                in_=wd[fb * P:(fb + 1) * P,
                                       dbg * 8 * P:(dbg + 1) * 8 * P])
                            for j in range(8):
                                nc.tensor.matmul(dps[j][:], wdt[:, j * P:(j + 1) * P],
                                                 ht[fb][:],
                                                 start=(fb == 0), stop=(fb == FB - 1))
                        for j in range(8):
                            db = dbg * 8 + j
                            yt = dsp.tile([P, TOK], F32, tag="y")
                            nc.vector.tensor_add(yt[:], dps[j][:], xTf[db][:])
                            nc.sync.dma_start(out=yT[db * P:(db + 1) * P, :], in_=yt[:])
        xf_cm.__exit__(None, None, None)
        attn_cm.__exit__(None, None, None)

    prune_redundant_waits(nc, verbose=True)
    nc.compile()
    return nc


# --------------------------------------------------------------------------
# Host orchestration
# --------------------------------------------------------------------------
_prog_cache = {}


def _get(key, builder, *args):
    if key not in _prog_cache:
        _prog_cache[key] = builder(*args)
    return _prog_cache[key]


def _bf(x):
    return np.ascontiguousarray(np.asarray(x, dtype=np.float32)).astype(
        ml_dtypes.bfloat16)


def _rope_tables(S, dim):
    freqs = 1.0 / (THETA ** (np.arange(0, dim, 2, dtype=np.float32) / dim))
    f = np.arange(S, dtype=np.float32)[:, None] * freqs[None, :]
    cos = np.repeat(np.cos(f), 2, axis=-1).astype(np.float32)
    sin = np.repeat(np.sin(f), 2, axis=-1).astype(np.float32)
    return cos, sin


def _rot_lhsT(dim):
    rt = np.zeros((dim, dim), np.float32)
    for i in range(dim // 2):
        rt[2 * i + 1, 2 * i] = -1.0
        rt[2 * i, 2 * i + 1] = 1.0
    return rt


def _timed_run(nc, in_maps, reps=100):
    """Execute on all cores with device-resident inputs; time warm reps.

    Returns (results_list, best_exec_seconds). Mimics
    bass2jax.run_bass_via_pjrt's multi-core path but keeps inputs on
    device so the timed region is pure NEFF execution + dispatch.
    """
    import time as _time
    import jax
    from jax.sharding import Mesh, PartitionSpec, NamedSharding
    from jax.experimental.shard_map import shard_map
    from concourse import bass2jax as b2j
    from concourse import mybir as _mb

    b2j.install_neuronx_cc_hook()
    n_cores = len(in_maps)
    in_names, out_names, out_avals, zero_outs = [], [], [], []
    for alloc in nc.m.functions[0].allocations:
        if not isinstance(alloc, _mb.MemoryLocationSet):
            continue
        name = alloc.memorylocations[0].name
        pid_name = nc.partition_id_tensor.name if nc.partition_id_tensor else None
        if alloc.kind == "ExternalInput":
            if name != pid_name:
                in_names.append(name)
        elif alloc.kind == "ExternalOutput":
            out_names.append(name)
            shape = tuple(alloc.tensor_shape)
            dtype = _mb.dt.np(alloc.dtype)
            out_avals.append(jax.core.ShapedArray(shape, dtype))
            zero_outs.append(np.zeros(shape, dtype))
    n_params = len(in_names)
    n_outs = len(out_avals)
    all_names = list(in_names) + list(out_names)
    if nc.partition_id_tensor is not None:
        all_names.append(nc.partition_id_tensor.name)

    def _body(*args):
        operands = list(args)
        if nc.partition_id_tensor is not None:
            operands.append(b2j.partition_id_tensor())
        outs = b2j._bass_exec_p.bind(
            *operands, out_avals=tuple(out_avals), in_names=tuple(all_names),
            out_names=tuple(out_names), lowering_input_output_aliases=(),
            sim_require_finite=True, sim_require_nnan=True, nc=nc)
        return tuple(outs)

    devices = jax.devices()[:n_cores]
    mesh = Mesh(np.asarray(devices), ("core",))
    donate = tuple(range(n_params, n_params + n_outs))
    sharded = jax.jit(
        shard_map(_body, mesh=mesh,
                  in_specs=(PartitionSpec("core"),) * (n_params + n_outs),
                  out_specs=(PartitionSpec("core"),) * n_outs,
                  check_rep=False),
        donate_argnums=donate, keep_unused=True)
    sh = NamedSharding(mesh, PartitionSpec("core"))
    dev_in = [jax.device_put(
        np.concatenate([np.asarray(in_maps[c][n]) for c in range(n_cores)], axis=0), sh)
        for n in in_names]
    # warmup + correctness outputs
    dz = [jax.device_put(
        np.zeros((n_cores * z.shape[0], *z.shape[1:]), z.dtype), sh)
        for z in zero_outs]
    jax.block_until_ready(dz)
    outs = sharded(*dev_in, *dz)
    jax.block_until_ready(outs)
    # pipelined timing: amortize per-call dispatch overhead over reps
    zsets = [[jax.device_put(
        np.zeros((n_cores * z.shape[0], *z.shape[1:]), z.dtype), sh)
        for z in zero_outs] for _ in range(reps)]
    jax.block_until_ready(zsets)
    t0 = _time.perf_counter()
    last = None
    for k in range(reps):
        last = sharded(*dev_in, *zsets[k])
    jax.block_until_ready(last)
    total = _time.perf_counter() - t0
    best = total / reps
    results = [
        {name: np.asarray(outs[i]).reshape(n_cores, *out_avals[i].shape)[c]
         for i, name in enumerate(out_names)}
        for c in range(n_cores)]
    return results, best


_last_exec_ns = []


class _Res:
    def __init__(self, results):
        self.results = results


def _run(nc, in_maps, trace=False):
    if trace:
        results, secs = _timed_run(nc, in_maps)
        _last_exec_ns.append(int(secs * 1e9))
        return _Res(results)
    res = run_bass_kernel_spmd(nc, in_maps, list(range(len(in_maps))))
    _last_exec_ns.append(res.exec_time_ns)
    return res


def kernel(query, key_value, g_q, g_kv, g_ffn, w_qc, w_kc, w_qr, w_kr, w_v,
           w_o, w_gate, w_up, w_down, _trace=True):
    query = np.asarray(query, np.float32)
    key_value = np.asarray(key_value, np.float32)
    Bq, Mq, _ = query.shape
    Nq = key_value.shape[1]
    HPC = N_H // NCORES
    NQ = Mq // TOK

    g_q = np.asarray(g_q, np.float32)[:, None]
    g_kv = np.asarray(g_kv, np.float32)[:, None]
    g_ffn = np.asarray(g_ffn, np.float32)[:, None]
    wqc = np.asarray(w_qc, np.float32) * g_q
    wqr_f = np.asarray(w_qr, np.float32) * g_q
    wkc = np.asarray(w_kc, np.float32) * g_kv
    wkr_f = np.asarray(w_kr, np.float32) * g_kv
    wv_f = np.asarray(w_v, np.float32) * g_kv
    wo_f = np.asarray(w_o, np.float32)
    wgate = _bf(np.asarray(w_gate, np.float32) * g_ffn)
    wup = _bf(np.asarray(w_up, np.float32) * g_ffn)
    wdown = _bf(np.asarray(w_down, np.float32))

    qT = _bf(query.transpose(0, 2, 1))
    kvT = _bf(key_value.transpose(0, 2, 1))
    cos, sin = _rope_tables(max(Mq, Nq), D_R)
    cos2T = np.ascontiguousarray(np.vstack([cos[:Mq].T] * HPC)).astype(np.float16)
    sin2T = np.ascontiguousarray(np.vstack([sin[:Mq].T] * HPC)).astype(np.float16)
    rot2T = _bf(np.kron(np.eye(HPC, dtype=np.float32), _rot_lhsT(D_R)))

    del _last_exec_ns[:]
    nc1 = _get(("fused", Bq, Mq, Nq, D, HPC), build_fused, Bq, Mq, Nq, D, HPC)
    EI = TOK // 2    # tokens per (core, batch): an eighth of each batch
    in_maps = []
    for c in range(NCORES):
        hs = slice(c * HPC * D_H, (c + 1) * HPC * D_H)
        rs = slice(c * HPC * D_R, (c + 1) * HPC * D_R)
        sl = slice(c * EI, (c + 1) * EI)
        qres = np.ascontiguousarray(
            np.concatenate([query[0, sl, :].T, query[1, sl, :].T], axis=1))
        in_maps.append({
            "qT": qT, "kvT": kvT,
            "wq": _bf(wqc[:, hs]), "wqr": _bf(wqr_f[:, rs]),
            "wk": _bf(wkc[:, hs]), "wkr": _bf(wkr_f[:, rs]),
            "wv": _bf(wv_f[:, hs]), "wo": _bf(wo_f[hs, :]),
            "cos2T": cos2T, "sin2T": sin2T, "rot2T": rot2T,
            "qres": qres, "wg": wgate, "wu": wup, "wd": wdown,
        })
    res = _run(nc1, in_maps, trace=_trace)

    y = np.empty((Bq, Mq, D), np.float32)
    for c in range(NCORES):
        sl = slice(c * EI, (c + 1) * EI)
        yT_c = res.results[c]["yT"]
        y[0, sl, :] = yT_c[:, :EI].T
        y[1, sl, :] = yT_c[:, EI:].T
    return y
